# revision 1
# baseline (speedup 1.0000x reference)
"""Causal self-attention Trainium2 kernel.

Problem: y = CausalSelfAttention(x) with B=4, T=2048, C=1024, H=16 heads,
head_dim D=64, qkv split order (k, q, v), softmax scale C**-0.5.

Sharding (8 cores): core = 2*b + g  -> batch b in 0..3, head-group g in 0..1
(8 heads per group).  Each core computes, for its batch and its 8 heads:
  qkv partial matmuls, causal attention, and the partial output projection
  y_partial = att_out @ W_proj[rows of this head group].
The host sums the two partial projections per batch (row-parallel tensor
parallelism reduced on host during unsharding).

Device layout notes (per core):
  xT    [128, 8, 2048]  x^T (C on partitions), loaded via DMA transpose (bf16)
  kqT   [128, 8, 2048]  (x @ W_kq)^T : blocks 0-3 = k-channels, 4-7 = q-channels
                         head h: 64*(h%2) partition offset, block h//2 (+4 for q)
  v_aug [128, 16, 520]  v in natural layout, 65 cols/head = [v(64) | ones(1)]
  S^T   [k partitions, q free] -> exp on ACT (scale 1/32, fp32 PSUM -> bf16)
        full 128k-blocks computed in per-head pairs (2-bank PSUM tile per
        pair); diagonal-band blocks col-sliced to valid columns and paired
        across the two concurrently-processed heads; the triangular mask is
        zeroed via gpsimd affine_select on the leading 128 columns.
  AV:   out^T[65, q] = [V|1]^T @ P^T accumulated over k tiles; row 64 = softmax
        denominator.  reciprocal (DVE) -> shift to partition 0 (DMA) ->
        partition_broadcast (gpsimd) -> multiply (DVE) -> place into att
        (SBUF->SBUF DMA, handles the odd-head partition offset).
  proj: y^T[1024, 2048] = W_proj_g(lhsT) @ att^T, streamed to HBM in fp32.

Scheduling: qkv for head-pair hp+1 is emitted interleaved with the attention
chunks of head-pair hp (separate PSUM tag) so the PE fills ACT-bound exp
windows with qkv matmuls.
"""

import numpy as np
import ml_dtypes

B, T, C, H = 4, 2048, 1024, 16
D = C // H          # 64
HPC = H // 2        # 8 heads per core
CG = C // 2         # 512 channels per head group
P = 128

_compiled = {}


def _build(t=T):
    import concourse.bacc as bacc
    import concourse.tile as tile
    import concourse.mybir as mybir

    f32 = mybir.dt.float32
    bf16 = mybir.dt.bfloat16
    Exp = mybir.ActivationFunctionType.Exp

    KT = C // P            # 8 contraction tiles over C
    MB = (2 * CG) // P     # 8 kq channel blocks (0-3 k, 4-7 q)
    TT = t // P            # token tiles of 128
    QC = t // 512          # q chunks of 512
    VB = CG // P           # 4 v/att channel blocks
    SCALE = float(C) ** -0.5

    nc = bacc.Bacc("TRN2", target_bir_lowering=False, debug=False,
                   num_devices=8)

    x_d = nc.dram_tensor("x", [t, C], bf16, kind="ExternalInput")
    wkq_d = nc.dram_tensor("wkq", [C, 2 * CG], bf16, kind="ExternalInput")
    wv_d = nc.dram_tensor("wv", [C, CG], bf16, kind="ExternalInput")
    wp_d = nc.dram_tensor("wp", [CG, C], bf16, kind="ExternalInput")
    y_d = nc.dram_tensor("y", [C, t], f32, kind="ExternalOutput")

    with tile.TileContext(nc) as tc:
        with (
            tc.tile_pool(name="persist", bufs=1) as persist,
            tc.tile_pool(name="psA", bufs=2, space="PSUM") as psA,
            tc.tile_pool(name="avP", bufs=1, space="PSUM") as avP,
            tc.tile_pool(name="ptP", bufs=16) as ptP,
            tc.tile_pool(name="ptdP", bufs=8) as ptdP,
            tc.tile_pool(name="rcP", bufs=2) as rcP,
            tc.tile_pool(name="rbP", bufs=2) as rbP,
            tc.tile_pool(name="atP", bufs=3) as atP,
            tc.tile_pool(name="yP", bufs=3) as yP,
        ):
            xT = persist.tile([P, KT, t], bf16)
            wkq_sb = persist.tile([P, KT, 2 * CG], bf16)
            wv_sb = persist.tile([P, KT, CG], bf16)
            wp_sb = persist.tile([P, VB, C], bf16)
            kqT = persist.tile([P, MB, t], bf16)
            v_aug = persist.tile([P, TT, HPC * (D + 1)], bf16)
            att = persist.tile([P, VB, t], bf16)

            dma_engs = [nc.sync, nc.sync]

            # PE warm-up: dependency-free matmuls run during the input-DMA
            # window so the HAM clock gate is at 8/8 when real work starts.
            wu_a = persist.tile([P, P], bf16)
            wu_b = persist.tile([P, 512], bf16)
            nc.vector.memset(wu_a, 0.0)
            nc.vector.memset(wu_b, 0.0)
            for _ in range(44):
                wps = psA.tile([P, 512], f32, name="wups", tag="qp", bufs=2)
                nc.tensor.matmul(wps, lhsT=wu_a, rhs=wu_b,
                                 start=True, stop=True,
                                 skip_group_check=True)

            # ---- loads: split across both HWDGE queues ----
            for ct in range(KT):
                dma_engs[ct % 2].dma_start(
                    xT[:, ct, :], x_d[:, ct * P:(ct + 1) * P], transpose=True)
            wkq_r = wkq_d.ap().rearrange("(kt p) m -> p kt m", p=P)
            wv_r = wv_d.ap().rearrange("(kt p) m -> p kt m", p=P)
            wp_r = wp_d.ap().rearrange("(kt p) m -> p kt m", p=P)
            for kt in range(KT):
                nc.sync.dma_start(wkq_sb[:, kt, :], wkq_r[:, kt, :])
                nc.sync.dma_start(wv_sb[:, kt, :], wv_r[:, kt, :])
            for kt in range(VB):
                nc.sync.dma_start(wp_sb[:, kt, :], wp_r[:, kt, :])
            nc.vector.memset(v_aug, 1.0)

            def chunk_pairs(n):
                return [list(range(i, min(i + 2, n))) for i in range(0, n, 2)]

            # one qkv "unit" = one PSUM accumulation group; "st"-tag units
            # use a 2-chunk (2-bank) tile, "qp"-tag units a 1-chunk tile
            def emit_kq_unit(mb, grp, tag):
                nu = 2 if tag == "st" else 1
                grp = grp if tag == "st" else grp[:1]
                ps = psA.tile([P, nu, 512], f32, name="ps", tag=tag,
                              bufs=2)
                for kt in range(KT):
                    for u, c in enumerate(grp):
                        nc.tensor.matmul(
                            ps[:, u, :],
                            lhsT=wkq_sb[:, kt, mb * P:(mb + 1) * P],
                            rhs=xT[:, kt, c * 512:(c + 1) * 512],
                            start=(kt == 0), stop=(kt == KT - 1),
                            skip_group_check=True)
                nc.vector.tensor_copy(
                    kqT[:, mb, grp[0] * 512:(grp[-1] + 1) * 512],
                    ps[:, 0:len(grp), :].rearrange("p u n -> p (u n)"))

            def emit_v_unit(grp, tag):
                nu = 2 if tag == "st" else 1
                grp = grp if tag == "st" else grp[:1]
                ps = psA.tile([P, nu, CG], f32, name="psv", tag=tag,
                              bufs=2)
                for kt in range(KT):
                    for u, tt in enumerate(grp):
                        nc.tensor.matmul(
                            ps[:, u, :],
                            lhsT=xT[:, kt, tt * P:(tt + 1) * P],
                            rhs=wv_sb[:, kt, :],
                            start=(kt == 0), stop=(kt == KT - 1),
                            skip_group_check=True)
                nc.vector.tensor_copy(
                    v_aug[:, grp[0]:grp[-1] + 1, :].rearrange(
                        "p u (h e) -> p u h e", e=D + 1)[:, :, :, 0:D],
                    ps[:, 0:len(grp), :].rearrange(
                        "p u (h d) -> p u h d", d=D))

            def emit_attn_chunk(hp, c):
                nfull = 4 * c
                avp = [avP.tile([D + 1, 512], f32, name=f"avp{hi}",
                                tag="avp", bufs=2)
                       for hi in range(2)]
                work = [[], []]
                for j in range(nfull):
                    # both heads' S^T for k-tile j in one 2-bank tile: the
                    # two matmuls are PE-adjacent with different row groups
                    # (rows 0:64 vs 64:128) so the systolic array overlaps
                    # them; one exp covers both heads
                    st = psA.tile([P, 2, 512], f32, name="st", tag="st")
                    for hi in range(2):
                        lo = D * hi
                        nc.tensor.matmul(
                            st[:, hi, :],
                            lhsT=kqT[lo:lo + D, hp, j * P:(j + 1) * P],
                            rhs=kqT[lo:lo + D, 4 + hp,
                                    c * 512:(c + 1) * 512],
                            start=True, stop=True,
                            skip_group_check=True)
                    pt = ptP.tile([P, 2, 512], bf16, name="pt", tag="pt")
                    nc.scalar.activation(pt, st, Exp, scale=SCALE)
                    for hi in range(2):
                        work[hi].append((pt[:, hi, :], j, 0))
                for dj in range(4):
                    j = nfull + dj
                    off = P * dj
                    w = 512 - off
                    st = psA.tile([P, 2, 512], f32, name="std", tag="st")
                    for hi in range(2):
                        lo = D * hi
                        nc.tensor.matmul(
                            st[:, hi, 0:w],
                            lhsT=kqT[lo:lo + D, hp, j * P:(j + 1) * P],
                            rhs=kqT[lo:lo + D, 4 + hp,
                                    c * 512 + off:(c + 1) * 512],
                            start=True, stop=True,
                            skip_group_check=True)
                    pt = ptdP.tile([P, 2, 512], bf16, name="ptd", tag="ptd")
                    nc.scalar.activation(pt[:, :, 0:w], st[:, :, 0:w],
                                         Exp, scale=SCALE)
                    nc.gpsimd.affine_select(
                        pt[:, :, 0:P], pt[:, :, 0:P],
                        pattern=[[0, 2], [1, P]],
                        compare_op=mybir.AluOpType.is_ge,
                        fill=0.0, base=0, channel_multiplier=-1)
                    for hi in range(2):
                        work[hi].append((pt[:, hi, 0:w], j, off))
                for hi in range(2):
                    h = 2 * hp + hi
                    n = len(work[hi])
                    for idx, (pap, j, off) in enumerate(work[hi]):
                        out_ap = avp[hi][:, off:512] if off else avp[hi]
                        nc.tensor.matmul(
                            out_ap,
                            lhsT=v_aug[:, j, h * (D + 1):(h + 1) * (D + 1)],
                            rhs=pap,
                            start=(idx == 0), stop=(idx == n - 1),
                            skip_group_check=True)
                    rc = rcP.tile([D + 1, 512], f32)
                    nc.vector.reciprocal(rc[D:D + 1, :], avp[hi][D:D + 1, :])
                    # partition_broadcast only reads physical partition 0;
                    # DMA-shift the reciprocal row there first.
                    rc0 = rcP.tile([1, 512], f32, name="rc0", tag="rc0")
                    nc.sync.dma_start(rc0, rc[D:D + 1, :])
                    rb = rbP.tile([D, 512], f32)
                    nc.gpsimd.partition_broadcast(rb, rc0[0:1, :], channels=D)
                    at = atP.tile([D, 512], bf16)
                    nc.vector.tensor_mul(at, avp[hi][0:D, :], rb)
                    nc.sync.dma_start(
                        att[D * hi:D * (hi + 1), hp, c * 512:(c + 1) * 512],
                        at)

            # ---- startup: just enough for attn(0, 0..1), alternate tags ----
            cps = chunk_pairs(QC)
            vps = chunk_pairs(TT)
            startup = [("kq", 0, cps[0]), ("kq", 4, cps[0])]
            startup += [("v", None, g) for g in vps[0:2]]
            for i, (kind, mb, grp) in enumerate(startup):
                if kind == "kq":
                    emit_kq_unit(mb, grp, "st")
                else:
                    emit_v_unit(grp, "st")

            # Remaining qkv/v units (single-chunk, 1-bank "qp" tiles),
            # emitted as PE filler between attention chunks.  Tile discovers
            # dependencies from TRACE order, so a producer MUST be emitted
            # before its first consumer chunk; each fill carries the global
            # chunk index it is first needed by.
            def cdiv(a, b):
                return -(-a // b)

            fills = []
            for tt in range(4, TT):
                # attn(0, c) AV reads v tiles tt <= 4c+3
                fills.append((max(0, cdiv(tt - 3, 4)), ("v", None, [tt])))
            for hp in range(4):
                for ck in range(QC):
                    if hp == 0 and ck in (0, 1):
                        continue
                    # k-side: attn(hp, c) reads j-tiles <= 4c+3 of block hp
                    fills.append((4 * hp + ck, ("kq", hp, [ck])))
                    # q-side: attn(hp, c) reads q chunk c of block 4+hp
                    fills.append((4 * hp + ck, ("kq", 4 + hp, [ck])))
            fills.sort(key=lambda f: f[0])

            # ---- attention with interleaved filler units ----
            nchunks = 4 * QC
            emitted = 0

            def emit_fills(upto):
                nonlocal emitted
                while emitted < min(upto, len(fills)):
                    _, (kind, mb, grp) = fills[emitted]
                    if kind == "kq":
                        emit_kq_unit(mb, grp, "qp")
                    else:
                        emit_v_unit(grp, "qp")
                    emitted += 1

            for hp in range(4):
                for c in range(QC):
                    ci = hp * QC + c
                    # everything this chunk reads must already be emitted
                    while emitted < len(fills) and fills[emitted][0] <= ci:
                        emit_fills(emitted + 1)
                    emit_attn_chunk(hp, c)
                    emit_fills(((ci + 4) * len(fills)) // nchunks)
            emit_fills(len(fills))

            # ---- projection: y^T = W_proj_g(lhsT) @ att^T ----
            # gi-major so the first-half chunks (ready before the final
            # attention chunks finish) are emitted first
            for grp in chunk_pairs(QC):
                for mb in range(C // P):
                    ps = psA.tile([P, 2, 512], f32, name="psp", tag="st",
                                  bufs=2)
                    for kt in range(VB):
                        for u, c in enumerate(grp):
                            nc.tensor.matmul(
                                ps[:, u, :],
                                lhsT=wp_sb[:, kt, mb * P:(mb + 1) * P],
                                rhs=att[:, kt, c * 512:(c + 1) * 512],
                                start=(kt == 0), stop=(kt == VB - 1),
                                skip_group_check=True)
                    yt = yP.tile([P, 2, 512], f32)
                    nc.vector.tensor_copy(yt[:, 0:len(grp), :],
                                          ps[:, 0:len(grp), :])
                    nc.sync.dma_start(
                        y_d[mb * P:(mb + 1) * P,
                            grp[0] * 512:(grp[-1] + 1) * 512],
                        yt[:, 0:len(grp), :].rearrange("p u n -> p (u n)"))

    nc.compile()
    return nc


def _get_compiled(t=T):
    if t not in _compiled:
        _compiled[t] = _build(t)
    return _compiled[t]


def make_in_maps(x, W_qkv, W_proj):
    bf = ml_dtypes.bfloat16
    x = np.asarray(x, dtype=np.float32)
    W_qkv = np.asarray(W_qkv, dtype=np.float32)
    W_proj = np.asarray(W_proj, dtype=np.float32)
    in_maps = []
    for core in range(8):
        b, g = core // 2, core % 2
        in_maps.append({
            "x": np.ascontiguousarray(x[b]).astype(bf),
            "wkq": np.concatenate(
                [W_qkv[:, g * CG:(g + 1) * CG],
                 W_qkv[:, C + g * CG:C + (g + 1) * CG]], axis=1).astype(bf),
            "wv": np.ascontiguousarray(
                W_qkv[:, 2 * C + g * CG:2 * C + (g + 1) * CG]).astype(bf),
            "wp": np.ascontiguousarray(
                W_proj[g * CG:(g + 1) * CG, :]).astype(bf),
        })
    return in_maps


def _run_axon_nodonate(nc, in_maps, n_cores=8):
    """Execute via PJRT/shard_map WITHOUT output-buffer donation.

    bass2jax.run_bass_via_pjrt donates the zero output operands; under the
    axon transport that donation intermittently corrupts multi-core results.
    This kernel writes every element of its output, so donation is not
    needed for correctness -- pass non-donated zero operands instead.
    """
    import jax
    from jax.sharding import Mesh, PartitionSpec
    from jax.experimental.shard_map import shard_map
    import concourse.mybir as mybir
    from concourse.bass2jax import _bass_exec_p, install_neuronx_cc_hook

    install_neuronx_cc_hook()
    in_names, out_names, out_avals = [], [], []
    for alloc in nc.m.functions[0].allocations:
        if not isinstance(alloc, mybir.MemoryLocationSet):
            continue
        name = alloc.memorylocations[0].name
        if alloc.kind == "ExternalInput":
            in_names.append(name)
        elif alloc.kind == "ExternalOutput":
            out_names.append(name)
            out_avals.append(jax.core.ShapedArray(
                tuple(alloc.tensor_shape), mybir.dt.np(alloc.dtype)))
    n_params = len(in_names)
    all_names = in_names + out_names
    pid_name = nc.partition_id_tensor.name if nc.partition_id_tensor else None

    def _body(*args):
        return tuple(_bass_exec_p.bind(
            *args,
            out_avals=tuple(out_avals),
            in_names=tuple(all_names),
            out_names=tuple(out_names),
            lowering_input_output_aliases=(),
            sim_require_finite=True,
            sim_require_nnan=True,
            nc=nc,
        ))

    devices = jax.devices()[:n_cores]
    mesh = Mesh(np.asarray(devices), ("core",))
    fn = jax.jit(
        shard_map(_body, mesh=mesh,
                  in_specs=(PartitionSpec("core"),) * (n_params + len(out_names)),
                  out_specs=(PartitionSpec("core"),) * len(out_names),
                  check_rep=False),
        keep_unused=True)
    concat_in = [
        np.concatenate([
            np.asarray(in_maps[c].get(
                nm, np.array([[c]], dtype=np.uint32) if nm == pid_name
                else None))
            for c in range(n_cores)], 0)
        for nm in in_names
    ]
    concat_zeros = [
        np.zeros((n_cores * a.shape[0], *a.shape[1:]), a.dtype)
        for a in out_avals
    ]
    out = fn(*concat_in, *concat_zeros)
    return [
        {nm: np.asarray(out[i]).reshape(n_cores, *out_avals[i].shape)[c]
         for i, nm in enumerate(out_names)}
        for c in range(n_cores)
    ]


def kernel(x, W_qkv, W_proj, _trace=False):
    from concourse._compat import axon_active

    nc = _get_compiled()
    in_maps = make_in_maps(x, W_qkv, W_proj)
    if axon_active():
        results = _run_axon_nodonate(nc, in_maps)
    else:
        import concourse.bass_utils as bass_utils
        res = bass_utils.run_bass_kernel_spmd(
            nc, in_maps, core_ids=list(range(8)), trace=_trace)
        if _trace:
            kernel.last_results = res
        results = res.results
    y = np.zeros((B, T, C), np.float32)
    for core in range(8):
        y[core // 2] += results[core]["y"].T
    return y



# revision 11
# speedup vs baseline: 1.3062x; 1.3062x over previous
"""Causal self-attention Trainium2 kernel (fp8 DoubleRow + AV-swap design).

Problem: y = CausalSelfAttention(x) with B=4, T=2048, C=1024, H=16 heads,
head_dim D=64, qkv split order (k, q, v), softmax scale C**-0.5.

Sharding (8 cores): core = 2*b + g  -> batch b in 0..3, head-group g in 0..1
(8 local heads per core).  Each core computes qkv for its 8 heads, causal
attention, and the partial projection y_partial = att_out @ W_proj[g rows].
The host sums the two partial projections per batch.

Key device-side structure (per core):
  kq:   fp8e4 DoubleRow matmuls (2 k-slices per instruction, 0.5 cyc/row).
        W_qkv columns are host-reordered so PSUM partitions land directly in
        the S-ready layout: block (side, quad, s2) holds d-channels
        [s2*32, s2*32+32) of heads 4*quad..4*quad+3 (lane-major).  W scaled
        by 32 on host so fp8 stays in normal range; exp scale divides by
        32*32.
  kqT:  [128, side, quad, s2, T] fp8 - head h lives on partitions
        32*(h%4)..+32 of quad h//4, with head-dim split across s2 in {0,1}.
  S:    per (head, j-tile) one fp8 DoubleRow matmul: lhsT [32, 2, 128] (k),
        rhs [32, 2, 512] (q chunk) -> S^T [128k, 512q] in PSUM (256 cyc).
  exp:  ACT, scale = C**-0.5/1024, bf16 out (pt tiles).  Full j-tiles
        batched in pairs; diagonal tiles column-sliced to the valid width
        and masked with gpsimd affine_select (leading 128 cols).
  AV:   transposed accumulation: out[q=128, 65] += pt_j[:, qslice]^T(lhsT)
        @ v_aug_j[128, 65](rhs, moving bf16) -> 65 cyc per instruction.
        Column 64 (ones in v_aug) accumulates the softmax denominator into
        the same partition as its q row.
  norm: DVE reciprocal [128, 4] + one broadcast tensor_mul per (pair, chunk,
        head) -> att_q [128q, qs, hi, 64] bf16.
  att:  one blocked DMA transpose per (pair, chunk): [128, 4*128] ->
        [128, 4, 128] producing channel-major att for the projection.
  proj: y^T[cout 128, q 512] = wp(lhsT) @ att(rhs, bf16) per (mb, chunk),
        emitted after each attention chunk-row completes (chunk-major loop)
        so projection overlaps the attention tail.

Scheduling: chunk-major (c outer, head-pair inner); kq/v units beyond the
startup set are emitted as PE filler between attention chunks (ACT is the
bottleneck engine; PE has slack).
"""

import numpy as np
import ml_dtypes

B, T, C, H = 4, 2048, 1024, 16
D = C // H          # 64
HPC = H // 2        # 8 heads per core
CG = C // 2         # 512 channels per head group
P = 128
WS = 32.0           # host-side W_qkv scale for fp8 range
SCALE = float(C) ** -0.5

_compiled = {}


def _build(t=T):
    import concourse.bacc as bacc
    import concourse.tile as tile
    import concourse.mybir as mybir

    f32 = mybir.dt.float32
    bf16 = mybir.dt.bfloat16
    f8 = mybir.dt.float8e4
    DR = mybir.MatmulPerfMode.DoubleRow
    Exp = mybir.ActivationFunctionType.Exp

    KT = C // P            # 8 contraction tiles over C
    KP = KT // 2           # 4 DoubleRow contraction pairs
    TT = t // P            # token tiles of 128
    QC = t // 512          # q chunks of 512
    VB = CG // P           # 4 att channel blocks (= head pairs)
    SCALE2 = SCALE / (WS * WS)

    nc = bacc.Bacc("TRN2", target_bir_lowering=False, debug=False,
                   num_devices=8)

    xf8_d = nc.dram_tensor("xf8", [C, t], f8, kind="ExternalInput")
    xf8l_d = nc.dram_tensor("xf8l", [C, t], f8, kind="ExternalInput")
    wkq_d = nc.dram_tensor("wkq", [C, C], f8, kind="ExternalInput")
    wvh_d = nc.dram_tensor("wvh", [C, CG], f8, kind="ExternalInput")
    wvl_d = nc.dram_tensor("wvl", [C, CG], f8, kind="ExternalInput")
    wp_d = nc.dram_tensor("wp", [CG, C], bf16, kind="ExternalInput")
    y_d = nc.dram_tensor("y", [C, t], f32, kind="ExternalOutput")

    with tile.TileContext(nc) as tc:
        with (
            tc.tile_pool(name="persist", bufs=1) as persist,
            tc.tile_pool(name="psS", bufs=2, space="PSUM") as psS,
            tc.tile_pool(name="avP", bufs=2, space="PSUM") as avP,
            tc.tile_pool(name="qpP", bufs=2, space="PSUM") as qpP,
            tc.tile_pool(name="ptP", bufs=32) as ptP,
            tc.tile_pool(name="rcP", bufs=4) as rcP,
            tc.tile_pool(name="atP", bufs=3) as atP,
            tc.tile_pool(name="yP", bufs=3) as yP,
        ):
            xf8 = persist.tile([P, KP, 2, t], f8)
            xf8l = persist.tile([P, KP, 2, t], f8)
            wkq_sb = persist.tile([P, KP, 2, C], f8)
            wvh_sb = persist.tile([P, KP, 2, CG], f8)
            wvl_sb = persist.tile([P, KP, 2, CG], f8)
            wp_sb = persist.tile([P, VB, C], bf16)
            # kqT[p, side(k/q), quad, s2, tok]
            kqT = persist.tile([P, 2, 2, 2, t], f8)
            v_aug = persist.tile([P, TT, HPC * (D + 1)], bf16)
            att = persist.tile([P, VB, t], bf16)

            # PE warm-up: dependency-free matmuls run during the input-DMA
            # window so the clock ramp is complete when real work starts.
            wu_a = persist.tile([P, P], bf16)
            wu_b = persist.tile([P, 512], bf16)
            nc.vector.memset(wu_a, 0.0)
            nc.vector.memset(wu_b, 0.0)
            for _ in range(10):
                wps = qpP.tile([P, 512], f32, name="wups", tag="qp", bufs=2)
                nc.tensor.matmul(wps, lhsT=wu_a, rhs=wu_b,
                                 start=True, stop=True,
                                 skip_group_check=True)

            # ---- loads, ordered for earliest compute start: wkq + x chunk 0
            # unblock the first kq units / S tiles; wv + xbf unblock v units
            # (first AV); wp is only needed by the first projection.
            xf8_r = xf8_d.ap().rearrange("(kp s p) t -> p kp s t", p=P, s=2)
            xf8l_r = xf8l_d.ap().rearrange(
                "(kp s p) t -> p kp s t", p=P, s=2)
            nc.sync.dma_start(
                wkq_sb,
                wkq_d.ap().rearrange("(kp s p) m -> p kp s m", p=P, s=2))
            nc.sync.dma_start(xf8[:, :, :, 0:512], xf8_r[:, :, :, 0:512])
            nc.sync.dma_start(
                wvh_sb, wvh_d.ap().rearrange("(kp s p) m -> p kp s m",
                                             p=P, s=2))
            nc.sync.dma_start(
                wvl_sb, wvl_d.ap().rearrange("(kp s p) m -> p kp s m",
                                             p=P, s=2))
            nc.sync.dma_start(xf8[:, :, :, 512:t], xf8_r[:, :, :, 512:t])
            nc.sync.dma_start(xf8l, xf8l_r)
            nc.sync.dma_start(
                wp_sb, wp_d.ap().rearrange("(kt p) m -> p kt m", p=P))
            # only the ones-columns need initialising; v columns are written
            # by the v units
            nc.vector.memset(
                v_aug.rearrange("p tt (h e) -> p tt h e", e=D + 1)[:, :, :, D],
                1.0)

            def emit_kq_unit(mb, c):
                # mb = (side, quad, s2) flat block index 0..7
                side, rem = divmod(mb, 4)
                quad, s2 = divmod(rem, 2)
                ps = qpP.tile([P, 512], f32, name="pskq", tag="qp", bufs=2)
                for kp in range(KP):
                    nc.tensor.matmul(
                        ps,
                        lhsT=wkq_sb[:, kp, :, mb * P:(mb + 1) * P],
                        rhs=xf8[:, kp, :, c * 512:(c + 1) * 512],
                        start=(kp == 0), stop=(kp == KP - 1),
                        perf_mode=DR, skip_group_check=True)
                nc.vector.tensor_copy(
                    kqT[:, side, quad, s2, c * 512:(c + 1) * 512], ps)

            def emit_v_unit(tt):
                ps = qpP.tile([P, CG], f32, name="psv", tag="qp", bufs=2)
                terms = [(xf8, wvh_sb), (xf8l, wvh_sb), (xf8, wvl_sb)]
                for ti, (xs, ws) in enumerate(terms):
                    for kp in range(KP):
                        nc.tensor.matmul(
                            ps,
                            lhsT=xs[:, kp, :, tt * P:(tt + 1) * P],
                            rhs=ws[:, kp, :, :],
                            start=(ti == 0 and kp == 0),
                            stop=(ti == 2 and kp == KP - 1),
                            perf_mode=DR, skip_group_check=True)
                nc.vector.tensor_copy(
                    v_aug[:, tt, :].rearrange(
                        "p (h e) -> p h e", e=D + 1)[:, :, 0:D],
                    ps.rearrange("p (h d) -> p h d", d=D))

            def emit_proj_unit(mb, c):
                ps = qpP.tile([P, 512], f32, name="psp", tag="qp", bufs=2)
                for kt in range(VB):
                    nc.tensor.matmul(
                        ps,
                        lhsT=wp_sb[:, kt, mb * P:(mb + 1) * P],
                        rhs=att[:, kt, c * 512:(c + 1) * 512],
                        start=(kt == 0), stop=(kt == VB - 1),
                        skip_group_check=True)
                yt = yP.tile([P, 512], f32)
                nc.vector.tensor_copy(yt, ps)
                nc.sync.dma_start(
                    y_d[mb * P:(mb + 1) * P, c * 512:(c + 1) * 512], yt)

            def emit_s_phase(hp, c):
                """S + exp (+ causal mask) for chunk (hp, c).  Returns a
                closure emitting the AV/normalise/transpose phase, so the
                main loop can software-pipeline: S of chunk n+1 is emitted
                before AV of chunk n, keeping the ACT engine fed while the
                PE runs AV and filler units."""
                quad = hp // 2
                lanes = (2 * (hp % 2), 2 * (hp % 2) + 1)
                heads = (2 * hp, 2 * hp + 1)

                def s_dr(out_ap, hi, j, w):
                    a = lanes[hi]
                    nc.tensor.matmul(
                        out_ap,
                        lhsT=kqT[32 * a:32 * a + 32, 0, quad, :,
                                 j * P:(j + 1) * P],
                        rhs=kqT[32 * a:32 * a + 32, 1, quad, :,
                                (c + 1) * 512 - w:(c + 1) * 512],
                        start=True, stop=True,
                        perf_mode=DR, skip_group_check=True,
                        tile_position=(32 * a, 0))

                # pt_slices[hi][j] -> AP covering q cols [off_j, 512) of the
                # exp'd S^T tile for (head hi, k-tile j), plus its offset
                pt_slices = [[None] * (4 * c + 4) for _ in range(2)]

                # full j-tiles, processed in (j, j+1) pairs per head
                for jp in range(2 * c):
                    for hi in range(2):
                        st = psS.tile([P, 2, 512], f32, name="st", tag="st")
                        for u in range(2):
                            s_dr(st[:, u, :], hi, 2 * jp + u, 512)
                        pt = ptP.tile([P, 2, 512], bf16, name="pt", tag="pt")
                        nc.scalar.activation(pt, st, Exp, scale=SCALE2)
                        for u in range(2):
                            pt_slices[hi][2 * jp + u] = (pt[:, u, :], 0)
                # diagonal j-tiles: both heads share a tile per dj
                for dj in range(4):
                    j = 4 * c + dj
                    w = 512 - P * dj
                    st = psS.tile([P, 2, 512], f32, name="std", tag="st")
                    for hi in range(2):
                        s_dr(st[:, hi, 0:w], hi, j, w)
                    pt = ptP.tile([P, 2, 512], bf16, name="ptd", tag="pt")
                    nc.scalar.activation(pt[:, :, 0:w], st[:, :, 0:w],
                                         Exp, scale=SCALE2)
                    nc.gpsimd.affine_select(
                        pt[:, :, 0:P], pt[:, :, 0:P],
                        pattern=[[0, 2], [1, P]],
                        compare_op=mybir.AluOpType.is_ge,
                        fill=0.0, base=0, channel_multiplier=-1)
                    for hi in range(2):
                        pt_slices[hi][j] = (pt[:, hi, 0:w], P * dj)

                def av_phase():
                    at_q = atP.tile([P, 4, 2, D], bf16)
                    for hi in range(2):
                        h = heads[hi]
                        av = avP.tile([P, 4, D + 1], f32, name=f"av{hi}",
                                      tag="av")
                        for qs in range(4):
                            njs = 4 * c + qs + 1
                            for j in range(njs):
                                pap, off = pt_slices[hi][j]
                                lo = qs * P - off
                                nc.tensor.matmul(
                                    av[:, qs, :],
                                    lhsT=pap[:, lo:lo + P],
                                    rhs=v_aug[:, j,
                                              h * (D + 1):(h + 1) * (D + 1)],
                                    start=(j == 0), stop=(j == njs - 1),
                                    skip_group_check=True)
                        rc = rcP.tile([P, 4], f32)
                        nc.vector.reciprocal(rc, av[:, :, D])
                        nc.vector.tensor_mul(
                            at_q[:, :, hi, :], av[:, :, 0:D],
                            rc[:, :, None].broadcast_to([P, 4, D]))
                    # blocked transpose: [128q, (qs hi d)] -> [128, qs, q]
                    nc.sync.dma_start(
                        att[:, hp, c * 512:(c + 1) * 512].rearrange(
                            "p (a b) -> p a b", b=P),
                        at_q, transpose=True)
                return av_phase

            # ---- startup: just enough for attn(hp=0, c=0) ----
            for mb in (0, 1, 4, 5):        # (k, quad0, s2=0/1), (q, quad0, ...)
                emit_kq_unit(mb, 0)
            for tt in range(4):
                emit_v_unit(tt)

            # Remaining kq/v units, emitted as PE filler between attention
            # chunks.  Tile discovers dependencies from TRACE order, so a
            # producer MUST be emitted before its first consumer chunk; each
            # fill carries the global chunk index (ci = 4*c + hp) it is first
            # needed by.
            def cdiv(a, b):
                return -(-a // b)

            fills = []
            for tt in range(4, TT):
                # attn(*, c) AV reads v tiles tt <= 4c+3 (exact need: the
                # early v units would stall the PE on the xf8l/wv loads)
                fills.append((4 * max(0, cdiv(tt - 3, 4)), ("v", tt)))
            for side in range(2):
                for quad in range(2):
                    for s2 in range(2):
                        mb = side * 4 + quad * 2 + s2
                        for cc in range(QC):
                            if mb in (0, 1, 4, 5) and cc == 0:
                                continue
                            # k side chunk cc needed by attn(2*quad, c>=cc);
                            # q side chunk cc needed by attn(2*quad, cc)
                            fills.append(
                                (max(0, 4 * cc + 2 * quad - 3),
                                 ("kq", mb, cc)))
            fills.sort(key=lambda f: f[0])

            nchunks = QC * 4
            emitted = 0

            def emit_fills(upto):
                nonlocal emitted
                while emitted < min(upto, len(fills)):
                    _, f = fills[emitted]
                    if f[0] == "kq":
                        emit_kq_unit(f[1], f[2])
                    else:
                        emit_v_unit(f[1])
                    emitted += 1

            # Software-pipelined schedule: S/exp of chunk n+1 is emitted
            # before the AV of chunk n (row 0 keeps all four S-phases ahead
            # so ACT covers the v-unit load window); projection of row c is
            # spread across the AV slots of row c+1.
            pending_av = []      # FIFO of av_phase closures
            pending_proj = []    # FIFO of (mb, c) projection units
            for c in range(QC):
                for hp in range(4):
                    ci = 4 * c + hp
                    # everything this chunk reads must already be emitted
                    while emitted < len(fills) and fills[emitted][0] <= ci:
                        emit_fills(emitted + 1)
                    pending_av.append(emit_s_phase(hp, c))
                    # lag taper: row 0 keeps all four S-phases ahead of the
                    # first AV (covers the v-unit load window), row 1 drains
                    # the backlog gradually, steady state keeps one chunk of
                    # S/exp in flight ahead of AV
                    lag = 4 if c == 0 else (max(1, 3 - hp) if c == 1 else 1)
                    while len(pending_av) > lag:
                        pending_av.pop(0)()
                        # projection of row cc is paced two rows behind (the
                        # ACT-slack rows); the last row takes double rate
                        nproj = 4 if c == QC - 1 else 2
                        for _ in range(nproj):
                            if pending_proj and (
                                    pending_proj[0][1] <= c - 2
                                    or c == QC - 1):
                                mb, cc = pending_proj.pop(0)
                                emit_proj_unit(mb, cc)
                    emit_fills(((ci + 3) * len(fills)) // nchunks)
                pending_proj.extend((mb, c) for mb in range(C // P))
            emit_fills(len(fills))
            while pending_av:
                pending_av.pop(0)()
            for mb, cc in pending_proj:
                emit_proj_unit(mb, cc)

    nc.compile()
    return nc


def _get_compiled(t=T):
    if t not in _compiled:
        _compiled[t] = _build(t)
    return _compiled[t]


def make_in_maps(x, W_qkv, W_proj):
    bf = ml_dtypes.bfloat16
    f8 = ml_dtypes.float8_e4m3
    x = np.asarray(x, dtype=np.float32)
    W_qkv = np.asarray(W_qkv, dtype=np.float32)
    W_proj = np.asarray(W_proj, dtype=np.float32)

    in_maps = []
    for core in range(8):
        b, g = core // 2, core % 2
        xT = np.ascontiguousarray(x[b].T)          # [C, T]
        # wkq column order: block (side, quad, s2): lane-major 32-channel
        # slices of heads 8g+4*quad..+3, d-range [32*s2, 32*s2+32)
        cols = []
        for side in range(2):
            base = side * C
            for quad in range(2):
                for s2 in range(2):
                    for lq in range(4):
                        h = 8 * g + 4 * quad + lq
                        st = base + h * D + s2 * 32
                        cols.append(np.arange(st, st + 32))
        cols = np.concatenate(cols)
        xf8 = xT.astype(f8)
        wvs = np.ascontiguousarray(
            W_qkv[:, 2 * C + g * CG:2 * C + (g + 1) * CG]) * WS
        wvh = wvs.astype(f8)
        in_maps.append({
            "xf8": xf8,
            "xf8l": (xT - xf8.astype(np.float32)).astype(f8),
            "wkq": (W_qkv[:, cols] * WS).astype(f8),
            "wvh": wvh,
            "wvl": (wvs - wvh.astype(np.float32)).astype(f8),
            # v (and hence att) carries the extra WS factor; fold the
            # inverse into the projection weights
            "wp": np.ascontiguousarray(
                W_proj[g * CG:(g + 1) * CG, :] / WS).astype(bf),
        })
    return in_maps


def _run_axon_nodonate(nc, in_maps, n_cores=8):
    """Execute via PJRT/shard_map WITHOUT output-buffer donation.

    bass2jax.run_bass_via_pjrt donates the zero output operands; under the
    axon transport that donation intermittently corrupts multi-core results.
    This kernel writes every element of its output, so donation is not
    needed for correctness -- pass non-donated zero operands instead.
    """
    import jax
    from jax.sharding import Mesh, PartitionSpec
    from jax.experimental.shard_map import shard_map
    import concourse.mybir as mybir
    from concourse.bass2jax import _bass_exec_p, install_neuronx_cc_hook

    install_neuronx_cc_hook()
    in_names, out_names, out_avals = [], [], []
    for alloc in nc.m.functions[0].allocations:
        if not isinstance(alloc, mybir.MemoryLocationSet):
            continue
        name = alloc.memorylocations[0].name
        if alloc.kind == "ExternalInput":
            in_names.append(name)
        elif alloc.kind == "ExternalOutput":
            out_names.append(name)
            out_avals.append(jax.core.ShapedArray(
                tuple(alloc.tensor_shape), mybir.dt.np(alloc.dtype)))
    n_params = len(in_names)
    all_names = in_names + out_names
    pid_name = nc.partition_id_tensor.name if nc.partition_id_tensor else None

    def _body(*args):
        return tuple(_bass_exec_p.bind(
            *args,
            out_avals=tuple(out_avals),
            in_names=tuple(all_names),
            out_names=tuple(out_names),
            lowering_input_output_aliases=(),
            sim_require_finite=True,
            sim_require_nnan=True,
            nc=nc,
        ))

    devices = jax.devices()[:n_cores]
    mesh = Mesh(np.asarray(devices), ("core",))
    fn = jax.jit(
        shard_map(_body, mesh=mesh,
                  in_specs=(PartitionSpec("core"),) * (n_params + len(out_names)),
                  out_specs=(PartitionSpec("core"),) * len(out_names),
                  check_rep=False),
        keep_unused=True)
    concat_in = [
        np.concatenate([
            np.asarray(in_maps[c].get(
                nm, np.array([[c]], dtype=np.uint32) if nm == pid_name
                else None))
            for c in range(n_cores)], 0)
        for nm in in_names
    ]
    concat_zeros = [
        np.zeros((n_cores * a.shape[0], *a.shape[1:]), a.dtype)
        for a in out_avals
    ]
    out = fn(*concat_in, *concat_zeros)
    return [
        {nm: np.asarray(out[i]).reshape(n_cores, *out_avals[i].shape)[c]
         for i, nm in enumerate(out_names)}
        for c in range(n_cores)
    ]


def kernel(x, W_qkv, W_proj, _trace=False):
    from concourse._compat import axon_active

    nc = _get_compiled()
    in_maps = make_in_maps(x, W_qkv, W_proj)
    if axon_active():
        results = _run_axon_nodonate(nc, in_maps)
    else:
        import concourse.bass_utils as bass_utils
        res = bass_utils.run_bass_kernel_spmd(
            nc, in_maps, core_ids=list(range(8)), trace=_trace)
        if _trace:
            kernel.last_results = res
        results = res.results
    y = np.zeros((B, T, C), np.float32)
    for core in range(8):
        y[core // 2] += results[core]["y"].T
    return y


# revision 31
# speedup vs baseline: 1.3685x; 1.0477x over previous
"""Causal self-attention Trainium2 kernel (fp8 DoubleRow + AV-swap design).

Problem: y = CausalSelfAttention(x) with B=4, T=2048, C=1024, H=16 heads,
head_dim D=64, qkv split order (k, q, v), softmax scale C**-0.5.

Sharding (8 cores): core = 2*b + g  -> batch b in 0..3, head-group g in 0..1
(8 local heads per core).  Each core computes qkv for its 8 heads, causal
attention, and the partial projection y_partial = att_out @ W_proj[g rows].
The host sums the two partial projections per batch.

Key device-side structure (per core):
  kq:   fp8e4 DoubleRow matmuls (2 k-slices per instruction, 0.5 cyc/row).
        W_qkv columns are host-reordered so PSUM partitions land directly in
        the S-ready layout: block (side, quad, s2) holds d-channels
        [s2*32, s2*32+32) of heads 4*quad..4*quad+3 (lane-major).  W scaled
        by 32 on host so fp8 stays in normal range; exp scale divides by
        32*32.
  kqT:  [128, side, quad, s2, T] fp8 - head h lives on partitions
        32*(h%4)..+32 of quad h//4, with head-dim split across s2 in {0,1}.
  S:    per (head, j-tile) one fp8 DoubleRow matmul: lhsT [32, 2, 128] (k),
        rhs [32, 2, 512] (q chunk) -> S^T [128k, 512q] in PSUM (256 cyc).
  exp:  ACT, scale = C**-0.5/1024, bf16 out (pt tiles).  Full j-tiles
        batched in pairs; diagonal tiles column-sliced to the valid width
        and masked with gpsimd affine_select (leading 128 cols).
  AV:   transposed accumulation: out[q=128, 65] += pt_j[:, qslice]^T(lhsT)
        @ v_aug_j[128, 65](rhs, moving bf16) -> 65 cyc per instruction.
        Column 64 (ones in v_aug) accumulates the softmax denominator into
        the same partition as its q row.
  norm: DVE reciprocal [128, 4] + one broadcast tensor_mul per (pair, chunk,
        head) -> att_q [128q, qs, hi, 64] bf16.
  att:  one blocked DMA transpose per (pair, chunk): [128, 4*128] ->
        [128, 4, 128] producing channel-major att for the projection.
  proj: y^T[cout 128, q 512] = wp(lhsT) @ att(rhs, bf16) per (mb, chunk),
        emitted after each attention chunk-row completes (chunk-major loop)
        so projection overlaps the attention tail.

Scheduling: chunk-major (c outer, head-pair inner); kq/v units beyond the
startup set are emitted as PE filler between attention chunks (ACT is the
bottleneck engine; PE has slack).
"""

import numpy as np
import ml_dtypes

B, T, C, H = 4, 2048, 1024, 16  # noqa
D = C // H          # 64
HPC = H // 2        # 8 heads per core
CG = C // 2         # 512 channels per head group
P = 128
WS = 32.0           # host-side W_qkv scale for fp8 range
SCALE = float(C) ** -0.5

_compiled = {}


def _build(t=T):
    import concourse.bacc as bacc
    import concourse.tile as tile
    import concourse.mybir as mybir

    f32 = mybir.dt.float32
    bf16 = mybir.dt.bfloat16
    f8 = mybir.dt.float8e4
    DR = mybir.MatmulPerfMode.DoubleRow
    Exp = mybir.ActivationFunctionType.Exp

    KT = C // P            # 8 contraction tiles over C
    KP = KT // 2           # 4 DoubleRow contraction pairs
    TT = t // P            # token tiles of 128
    QC = t // 512          # q chunks of 512
    VB = CG // P           # 4 att channel blocks (= head pairs)
    SCALE2 = SCALE / (WS * WS)

    nc = bacc.Bacc("TRN2", target_bir_lowering=False, debug=False,
                   num_devices=8)

    xf8_d = nc.dram_tensor("xf8", [C, t], f8, kind="ExternalInput")
    xf8l_d = nc.dram_tensor("xf8l", [C, t], f8, kind="ExternalInput")
    wkq_d = nc.dram_tensor("wkq", [C, C], f8, kind="ExternalInput")
    wvh_d = nc.dram_tensor("wvh", [C, CG], f8, kind="ExternalInput")
    wvl_d = nc.dram_tensor("wvl", [C, CG], f8, kind="ExternalInput")
    wp_d = nc.dram_tensor("wp", [CG, C], bf16, kind="ExternalInput")
    y_d = nc.dram_tensor("y", [C, t], f32, kind="ExternalOutput")

    with tile.TileContext(nc) as tc:
        with (
            tc.tile_pool(name="persist", bufs=1) as persist,
            tc.tile_pool(name="psS", bufs=2, space="PSUM") as psS,
            tc.tile_pool(name="avP", bufs=2, space="PSUM") as avP,
            tc.tile_pool(name="qpP", bufs=2, space="PSUM") as qpP,
            tc.tile_pool(name="ptP", bufs=32) as ptP,
            tc.tile_pool(name="rcP", bufs=4) as rcP,
            tc.tile_pool(name="atP", bufs=3) as atP,
            tc.tile_pool(name="yP", bufs=3) as yP,
        ):
            xf8 = persist.tile([P, KP, 2, t], f8)
            xf8l = persist.tile([P, KP, 2, t], f8)
            wkq_sb = persist.tile([P, KP, 2, C], f8)
            wvh_sb = persist.tile([P, KP, 2, CG], f8)
            wvl_sb = persist.tile([P, KP, 2, CG], f8)
            wp_sb = persist.tile([P, VB, C], bf16)
            # kqT[p, side(k/q), quad, s2, tok]
            kqT = persist.tile([P, 2, 2, 2, t], f8)
            v_aug = persist.tile([P, TT, HPC * (D + 1)], bf16)
            att = persist.tile([P, VB, t], bf16)

            # PE warm-up: dependency-free matmuls run during the input-DMA
            # window so the clock ramp is complete when real work starts.
            wu_a = persist.tile([P, P], bf16)
            wu_b = persist.tile([P, 512], bf16)
            nc.vector.memset(wu_a, 0.0)
            nc.vector.memset(wu_b, 0.0)
            for _ in range(10):
                wps = qpP.tile([P, 512], f32, name="wups", tag="qp", bufs=2)
                nc.tensor.matmul(wps, lhsT=wu_a, rhs=wu_b,
                                 start=True, stop=True,
                                 skip_group_check=True)

            # ---- loads, ordered for earliest compute start: wkq + x chunk 0
            # unblock the first kq units / S tiles; wv + xbf unblock v units
            # (first AV); wp is only needed by the first projection.
            xf8_r = xf8_d.ap().rearrange("(kp s p) t -> p kp s t", p=P, s=2)
            xf8l_r = xf8l_d.ap().rearrange(
                "(kp s p) t -> p kp s t", p=P, s=2)
            wkq_r = wkq_d.ap().rearrange("(kp s p) m -> p kp s m",
                                         p=P, s=2)
            # startup (quad 0, chunk 0) needs wkq blocks 0,1 (cols 0:256)
            # and 4,5 (cols 512:768) plus x chunk 0; v units for chunk c
            # need only the token-chunk-c slices of xf8/xf8l
            nc.sync.dma_start(wkq_sb[:, :, :, 0:512], wkq_r[:, :, :, 0:512])
            nc.sync.dma_start(xf8[:, :, :, 0:512], xf8_r[:, :, :, 0:512])
            nc.sync.dma_start(
                wvh_sb, wvh_d.ap().rearrange("(kp s p) m -> p kp s m",
                                             p=P, s=2))
            nc.sync.dma_start(
                wvl_sb, wvl_d.ap().rearrange("(kp s p) m -> p kp s m",
                                             p=P, s=2))
            nc.sync.dma_start(xf8l[:, :, :, 0:512], xf8l_r[:, :, :, 0:512])
            nc.sync.dma_start(wkq_sb[:, :, :, 512:1024],
                              wkq_r[:, :, :, 512:1024])
            nc.sync.dma_start(xf8[:, :, :, 512:t], xf8_r[:, :, :, 512:t])
            nc.sync.dma_start(xf8l[:, :, :, 512:t], xf8l_r[:, :, :, 512:t])
            nc.sync.dma_start(
                wp_sb, wp_d.ap().rearrange("(kt p) m -> p kt m", p=P))
            # only the ones-columns need initialising; v columns are written
            # by the v units
            nc.vector.memset(
                v_aug.rearrange("p tt (h e) -> p tt h e", e=D + 1)[:, :, :, D],
                1.0)

            # column position of block mb in the host-reordered wkq
            WKQ_POS = {0: 0, 1: 1, 4: 2, 5: 3, 2: 4, 3: 5, 6: 6, 7: 7}

            def emit_kq_unit(mb, c):
                # mb = (side, quad, s2) flat block index 0..7
                side, rem = divmod(mb, 4)
                quad, s2 = divmod(rem, 2)
                pos = WKQ_POS[mb]
                ps = qpP.tile([P, 512], f32, name="pskq", tag="qp", bufs=2)
                for kp in range(KP):
                    nc.tensor.matmul(
                        ps,
                        lhsT=wkq_sb[:, kp, :, pos * P:(pos + 1) * P],
                        rhs=xf8[:, kp, :, c * 512:(c + 1) * 512],
                        start=(kp == 0), stop=(kp == KP - 1),
                        perf_mode=DR, skip_group_check=True)
                nc.vector.tensor_copy(
                    kqT[:, side, quad, s2, c * 512:(c + 1) * 512], ps)

            def emit_v_unit(tt):
                ps = qpP.tile([P, CG], f32, name="psv", tag="qp", bufs=2)
                terms = [(xf8, wvh_sb), (xf8l, wvh_sb), (xf8, wvl_sb)]
                for ti, (xs, ws) in enumerate(terms):
                    for kp in range(KP):
                        nc.tensor.matmul(
                            ps,
                            lhsT=xs[:, kp, :, tt * P:(tt + 1) * P],
                            rhs=ws[:, kp, :, :],
                            start=(ti == 0 and kp == 0),
                            stop=(ti == 2 and kp == KP - 1),
                            perf_mode=DR, skip_group_check=True)
                nc.vector.tensor_copy(
                    v_aug[:, tt, :].rearrange(
                        "p (h e) -> p h e", e=D + 1)[:, :, 0:D],
                    ps.rearrange("p (h d) -> p h d", d=D))

            def emit_proj_unit(mb, c):
                ps = qpP.tile([P, 512], f32, name="psp", tag="qp", bufs=2)
                for kt in range(VB):
                    nc.tensor.matmul(
                        ps,
                        lhsT=wp_sb[:, kt, mb * P:(mb + 1) * P],
                        rhs=att[:, kt, c * 512:(c + 1) * 512],
                        start=(kt == 0), stop=(kt == VB - 1),
                        skip_group_check=True)
                yt = yP.tile([P, 512], f32)
                nc.vector.tensor_copy(yt, ps)
                nc.sync.dma_start(
                    y_d[mb * P:(mb + 1) * P, c * 512:(c + 1) * 512], yt)

            def emit_s_phase(hp, c, fuse_av=False):
                """S + exp (+ causal mask) for chunk (hp, c).  Returns a
                closure emitting the AV/normalise/transpose phase, so the
                main loop can software-pipeline: S of chunk n+1 is emitted
                before AV of chunk n, keeping the ACT engine fed while the
                PE runs AV and filler units.  With fuse_av (final chunk),
                each AV qs-group is emitted right after its diagonal tile so
                the kernel tail shrinks; returns None."""
                quad = hp // 2
                lanes = (2 * (hp % 2), 2 * (hp % 2) + 1)
                heads = (2 * hp, 2 * hp + 1)

                def s_dr(out_ap, hi, j, w):
                    a = lanes[hi]
                    nc.tensor.matmul(
                        out_ap,
                        lhsT=kqT[32 * a:32 * a + 32, 0, quad, :,
                                 j * P:(j + 1) * P],
                        rhs=kqT[32 * a:32 * a + 32, 1, quad, :,
                                (c + 1) * 512 - w:(c + 1) * 512],
                        start=True, stop=True,
                        perf_mode=DR, skip_group_check=True,
                        tile_position=(32 * a, 0))

                # pt_slices[hi][j] -> AP covering q cols [off_j, 512) of the
                # exp'd S^T tile for (head hi, k-tile j), plus its offset
                pt_slices = [[None] * (4 * c + 4) for _ in range(2)]

                # full j-tiles, processed in (j, j+1) pairs per head
                for jp in range(2 * c):
                    for hi in range(2):
                        st = psS.tile([P, 2, 512], f32, name="st", tag="st")
                        for u in range(2):
                            s_dr(st[:, u, :], hi, 2 * jp + u, 512)
                        pt = ptP.tile([P, 2, 512], bf16, name="pt", tag="pt")
                        nc.scalar.activation(pt, st, Exp, scale=SCALE2)
                        for u in range(2):
                            pt_slices[hi][2 * jp + u] = (pt[:, u, :], 0)
                def emit_av_group(av, hi, qs):
                    h = heads[hi]
                    njs = 4 * c + qs + 1
                    for j in range(njs):
                        pap, off = pt_slices[hi][j]
                        lo = qs * P - off
                        nc.tensor.matmul(
                            av[:, qs, :],
                            lhsT=pap[:, lo:lo + P],
                            rhs=v_aug[:, j,
                                      h * (D + 1):(h + 1) * (D + 1)],
                            start=(j == 0), stop=(j == njs - 1),
                            skip_group_check=True)

                def norm_store(at_q, avs):
                    for hi in range(2):
                        rc = rcP.tile([P, 4], f32)
                        nc.vector.reciprocal(rc, avs[hi][:, :, D])
                        nc.vector.tensor_mul(
                            at_q[:, :, hi, :], avs[hi][:, :, 0:D],
                            rc[:, :, None].broadcast_to([P, 4, D]))
                    # blocked transpose: [128q, (qs hi d)] -> [128, qs, q]
                    nc.sync.dma_start(
                        att[:, hp, c * 512:(c + 1) * 512].rearrange(
                            "p (a b) -> p a b", b=P),
                        at_q, transpose=True)

                if fuse_av:
                    avs = [avP.tile([P, 4, D + 1], f32, name=f"av{hi}",
                                    tag="av") for hi in range(2)]

                # diagonal j-tiles: both heads share a tile per dj
                for dj in range(4):
                    j = 4 * c + dj
                    w = 512 - P * dj
                    st = psS.tile([P, 2, 512], f32, name="std", tag="st")
                    for hi in range(2):
                        s_dr(st[:, hi, 0:w], hi, j, w)
                    pt = ptP.tile([P, 2, 512], bf16, name="ptd", tag="pt")
                    nc.scalar.activation(pt[:, :, 0:w], st[:, :, 0:w],
                                         Exp, scale=SCALE2)
                    nc.gpsimd.affine_select(
                        pt[:, :, 0:P], pt[:, :, 0:P],
                        pattern=[[0, 2], [1, P]],
                        compare_op=mybir.AluOpType.is_ge,
                        fill=0.0, base=0, channel_multiplier=-1)
                    for hi in range(2):
                        pt_slices[hi][j] = (pt[:, hi, 0:w], P * dj)
                    if fuse_av:
                        for hi in range(2):
                            emit_av_group(avs[hi], hi, dj)

                if fuse_av:
                    at_q = atP.tile([P, 4, 2, D], bf16)
                    norm_store(at_q, avs)
                    return None

                def av_phase():
                    at_q = atP.tile([P, 4, 2, D], bf16)
                    avs = []
                    for hi in range(2):
                        av = avP.tile([P, 4, D + 1], f32, name=f"av{hi}",
                                      tag="av")
                        avs.append(av)
                        for qs in range(4):
                            emit_av_group(av, hi, qs)
                    norm_store(at_q, avs)
                return av_phase

            # ---- startup: just enough for attn(hp=0, c=0) ----
            for mb in (0, 1, 4, 5):        # (k, quad0, s2=0/1), (q, quad0, ...)
                emit_kq_unit(mb, 0)
            for tt in range(4):
                emit_v_unit(tt)

            # Remaining kq/v units, emitted as PE filler between attention
            # chunks.  Tile discovers dependencies from TRACE order, so a
            # producer MUST be emitted before its first consumer chunk; each
            # fill carries the global chunk index (ci = 4*c + hp) it is first
            # needed by.
            def cdiv(a, b):
                return -(-a // b)

            fills = []
            for tt in range(4, TT):
                # attn(*, c) AV reads v tiles tt <= 4c+3 (exact need: the
                # early v units would stall the PE on the xf8l/wv loads)
                fills.append(
                    ((4 * max(0, cdiv(tt - 3, 4)), 0), ("v", tt)))
            for side in range(2):
                for quad in range(2):
                    for s2 in range(2):
                        mb = side * 4 + quad * 2 + s2
                        for cc in range(QC):
                            if mb in (0, 1, 4, 5) and cc == 0:
                                continue
                            # k side chunk cc needed by attn(2*quad, c>=cc);
                            # q side chunk cc needed by attn(2*quad, cc)
                            fills.append(
                                ((4 * cc + 2 * quad, 1), ("kq", mb, cc)))
            fills.sort(key=lambda f: f[0])

            nchunks = QC * 4
            emitted = 0

            def emit_fills(upto):
                nonlocal emitted
                while emitted < min(upto, len(fills)):
                    _, f = fills[emitted]
                    if f[0] == "kq":
                        emit_kq_unit(f[1], f[2])
                    else:
                        emit_v_unit(f[1])
                    emitted += 1

            # Software-pipelined schedule: S/exp of chunk n+1 is emitted
            # before the AV of chunk n (row 0 keeps all four S-phases ahead
            # so ACT covers the v-unit load window); projection of row c is
            # spread across the AV slots of row c+1.
            pending_av = []      # FIFO of av_phase closures
            pending_proj = []    # FIFO of (mb, c) projection units
            for c in range(QC):
                for hp in range(4):
                    ci = 4 * c + hp
                    # everything this chunk reads must already be emitted
                    while emitted < len(fills) and fills[emitted][0][0] <= ci:
                        emit_fills(emitted + 1)
                    pending_av.append(emit_s_phase(hp, c))
                    # lag taper: row 0 keeps all four S-phases ahead of the
                    # first AV (covers the v-unit load window), row 1 drains
                    # the backlog gradually, steady state keeps one chunk of
                    # S/exp in flight ahead of AV
                    lag = (4 if c == 0 else
                           max(1, 3 - hp) if c == 1 else 1)
                    while len(pending_av) > lag:
                        pending_av.pop(0)()
                        # projection of row cc is paced two rows behind (the
                        # ACT-slack rows); the last row takes double rate
                        nproj = 4 if c == QC - 1 else 2
                        for _ in range(nproj):
                            if pending_proj and (
                                    pending_proj[0][1] <= c - 2
                                    or c == QC - 1):
                                mb, cc = pending_proj.pop(0)
                                emit_proj_unit(mb, cc)
                    emit_fills(((ci + 3) * len(fills)) // nchunks)
                pending_proj.extend((mb, c) for mb in range(C // P))
            emit_fills(len(fills))
            while pending_av:
                pending_av.pop(0)()
            for mb, cc in pending_proj:
                emit_proj_unit(mb, cc)

    nc.compile()
    return nc


def _get_compiled(t=T):
    if t not in _compiled:
        _compiled[t] = _build(t)
    return _compiled[t]


def make_in_maps(x, W_qkv, W_proj):
    bf = ml_dtypes.bfloat16
    f8 = ml_dtypes.float8_e4m3
    x = np.asarray(x, dtype=np.float32)
    W_qkv = np.asarray(W_qkv, dtype=np.float32)
    W_proj = np.asarray(W_proj, dtype=np.float32)

    in_maps = []
    for core in range(8):
        b, g = core // 2, core % 2
        xT = np.ascontiguousarray(x[b].T)          # [C, T]
        # wkq column order: block (side, quad, s2): lane-major 32-channel
        # slices of heads 8g+4*quad..+3, d-range [32*s2, 32*s2+32)
        cols = []
        for side, quad in ((0, 0), (1, 0), (0, 1), (1, 1)):
            base = side * C
            for s2 in range(2):
                for lq in range(4):
                    h = 8 * g + 4 * quad + lq
                    st = base + h * D + s2 * 32
                    cols.append(np.arange(st, st + 32))
        cols = np.concatenate(cols)
        xf8 = xT.astype(f8)
        wvs = np.ascontiguousarray(
            W_qkv[:, 2 * C + g * CG:2 * C + (g + 1) * CG]) * WS
        wvh = wvs.astype(f8)
        in_maps.append({
            "xf8": xf8,
            "xf8l": (xT - xf8.astype(np.float32)).astype(f8),
            "wkq": (W_qkv[:, cols] * WS).astype(f8),
            "wvh": wvh,
            "wvl": (wvs - wvh.astype(np.float32)).astype(f8),
            # v (and hence att) carries the extra WS factor; fold the
            # inverse into the projection weights
            "wp": np.ascontiguousarray(
                W_proj[g * CG:(g + 1) * CG, :] / WS).astype(bf),
        })
    return in_maps


def _run_axon_nodonate(nc, in_maps, n_cores=8):
    """Execute via PJRT/shard_map WITHOUT output-buffer donation.

    bass2jax.run_bass_via_pjrt donates the zero output operands; under the
    axon transport that donation intermittently corrupts multi-core results.
    This kernel writes every element of its output, so donation is not
    needed for correctness -- pass non-donated zero operands instead.
    """
    import jax
    from jax.sharding import Mesh, PartitionSpec
    from jax.experimental.shard_map import shard_map
    import concourse.mybir as mybir
    from concourse.bass2jax import _bass_exec_p, install_neuronx_cc_hook

    install_neuronx_cc_hook()
    in_names, out_names, out_avals = [], [], []
    for alloc in nc.m.functions[0].allocations:
        if not isinstance(alloc, mybir.MemoryLocationSet):
            continue
        name = alloc.memorylocations[0].name
        if alloc.kind == "ExternalInput":
            in_names.append(name)
        elif alloc.kind == "ExternalOutput":
            out_names.append(name)
            out_avals.append(jax.core.ShapedArray(
                tuple(alloc.tensor_shape), mybir.dt.np(alloc.dtype)))
    n_params = len(in_names)
    all_names = in_names + out_names
    pid_name = nc.partition_id_tensor.name if nc.partition_id_tensor else None

    def _body(*args):
        return tuple(_bass_exec_p.bind(
            *args,
            out_avals=tuple(out_avals),
            in_names=tuple(all_names),
            out_names=tuple(out_names),
            lowering_input_output_aliases=(),
            sim_require_finite=True,
            sim_require_nnan=True,
            nc=nc,
        ))

    devices = jax.devices()[:n_cores]
    mesh = Mesh(np.asarray(devices), ("core",))
    fn = jax.jit(
        shard_map(_body, mesh=mesh,
                  in_specs=(PartitionSpec("core"),) * (n_params + len(out_names)),
                  out_specs=(PartitionSpec("core"),) * len(out_names),
                  check_rep=False),
        keep_unused=True)
    concat_in = [
        np.concatenate([
            np.asarray(in_maps[c].get(
                nm, np.array([[c]], dtype=np.uint32) if nm == pid_name
                else None))
            for c in range(n_cores)], 0)
        for nm in in_names
    ]
    concat_zeros = [
        np.zeros((n_cores * a.shape[0], *a.shape[1:]), a.dtype)
        for a in out_avals
    ]
    out = fn(*concat_in, *concat_zeros)
    return [
        {nm: np.asarray(out[i]).reshape(n_cores, *out_avals[i].shape)[c]
         for i, nm in enumerate(out_names)}
        for c in range(n_cores)
    ]


def kernel(x, W_qkv, W_proj, _trace=False):
    from concourse._compat import axon_active

    nc = _get_compiled()
    in_maps = make_in_maps(x, W_qkv, W_proj)
    if axon_active():
        results = _run_axon_nodonate(nc, in_maps)
    else:
        import concourse.bass_utils as bass_utils
        res = bass_utils.run_bass_kernel_spmd(
            nc, in_maps, core_ids=list(range(8)), trace=_trace)
        if _trace:
            kernel.last_results = res
        results = res.results
    y = np.zeros((B, T, C), np.float32)
    for core in range(8):
        y[core // 2] += results[core]["y"].T
    return y


# revision 33
# speedup vs baseline: 1.4242x; 1.0407x over previous
"""Causal self-attention Trainium2 kernel (fp8 DoubleRow + AV-swap design).

Problem: y = CausalSelfAttention(x) with B=4, T=2048, C=1024, H=16 heads,
head_dim D=64, qkv split order (k, q, v), softmax scale C**-0.5.

Sharding (8 cores): core = 2*b + g  -> batch b in 0..3, head-group g in 0..1
(8 local heads per core).  Each core computes qkv for its 8 heads, causal
attention, and the partial projection y_partial = att_out @ W_proj[g rows].
The host sums the two partial projections per batch.

Key device-side structure (per core):
  kq:   fp8e4 DoubleRow matmuls (2 k-slices per instruction, 0.5 cyc/row).
        W_qkv columns are host-reordered so PSUM partitions land directly in
        the S-ready layout: block (side, quad, s2) holds d-channels
        [s2*32, s2*32+32) of heads 4*quad..4*quad+3 (lane-major).  W scaled
        by 32 on host so fp8 stays in normal range; exp scale divides by
        32*32.
  kqT:  [128, side, quad, s2, T] fp8 - head h lives on partitions
        32*(h%4)..+32 of quad h//4, with head-dim split across s2 in {0,1}.
  S:    per (head, j-tile) one fp8 DoubleRow matmul: lhsT [32, 2, 128] (k),
        rhs [32, 2, 512] (q chunk) -> S^T [128k, 512q] in PSUM (256 cyc).
  exp:  ACT, scale = C**-0.5/1024, bf16 out (pt tiles).  Full j-tiles
        batched in pairs; diagonal tiles column-sliced to the valid width
        and masked with gpsimd affine_select (leading 128 cols).
  AV:   transposed accumulation: out[q=128, 65] += pt_j[:, qslice]^T(lhsT)
        @ v_aug_j[128, 65](rhs, moving bf16) -> 65 cyc per instruction.
        Column 64 (ones in v_aug) accumulates the softmax denominator into
        the same partition as its q row.
  norm: DVE reciprocal [128, 4] + one broadcast tensor_mul per (pair, chunk,
        head) -> att_q [128q, qs, hi, 64] bf16.
  att:  one blocked DMA transpose per (pair, chunk): [128, 4*128] ->
        [128, 4, 128] producing channel-major att for the projection.
  proj: y^T[cout 128, q 512] = wp(lhsT) @ att(rhs, bf16) per (mb, chunk),
        emitted after each attention chunk-row completes (chunk-major loop)
        so projection overlaps the attention tail.

Scheduling: chunk-major (c outer, head-pair inner); kq/v units beyond the
startup set are emitted as PE filler between attention chunks (ACT is the
bottleneck engine; PE has slack).
"""

import numpy as np
import ml_dtypes

B, T, C, H = 4, 2048, 1024, 16  # noqa
D = C // H          # 64
HPC = H // 2        # 8 heads per core
CG = C // 2         # 512 channels per head group
P = 128
WS = 32.0           # host-side W_qkv scale for fp8 range
SCALE = float(C) ** -0.5

_compiled = {}


def _build(t=T):
    import concourse.bacc as bacc
    import concourse.tile as tile
    import concourse.mybir as mybir

    f32 = mybir.dt.float32
    bf16 = mybir.dt.bfloat16
    f8 = mybir.dt.float8e4
    DR = mybir.MatmulPerfMode.DoubleRow
    Exp = mybir.ActivationFunctionType.Exp

    KT = C // P            # 8 contraction tiles over C
    KP = KT // 2           # 4 DoubleRow contraction pairs
    TT = t // P            # token tiles of 128
    QC = t // 512          # q chunks of 512
    VB = CG // P           # 4 att channel blocks (= head pairs)
    SCALE2 = SCALE / (WS * WS)

    nc = bacc.Bacc("TRN2", target_bir_lowering=False, debug=False,
                   num_devices=8)

    xf8_d = nc.dram_tensor("xf8", [C, t], f8, kind="ExternalInput")
    xf8l_d = nc.dram_tensor("xf8l", [C, t], f8, kind="ExternalInput")
    wkq_d = nc.dram_tensor("wkq", [C, C], f8, kind="ExternalInput")
    wvh_d = nc.dram_tensor("wvh", [C, CG], f8, kind="ExternalInput")
    wvl_d = nc.dram_tensor("wvl", [C, CG], f8, kind="ExternalInput")
    wp_d = nc.dram_tensor("wp", [CG, C], bf16, kind="ExternalInput")
    y_d = nc.dram_tensor("y", [C, t], f32, kind="ExternalOutput")

    with tile.TileContext(nc) as tc:
        with (
            tc.tile_pool(name="persist", bufs=1) as persist,
            tc.tile_pool(name="psS", bufs=2, space="PSUM") as psS,
            tc.tile_pool(name="avP", bufs=2, space="PSUM") as avP,
            tc.tile_pool(name="qpP", bufs=2, space="PSUM") as qpP,
            tc.tile_pool(name="ptP", bufs=32) as ptP,
            tc.tile_pool(name="rcP", bufs=4) as rcP,
            tc.tile_pool(name="atP", bufs=3) as atP,
            tc.tile_pool(name="yP", bufs=3) as yP,
        ):
            xf8 = persist.tile([P, KP, 2, t], f8)
            xf8l = persist.tile([P, KP, 2, t], f8)
            wkq_sb = persist.tile([P, KP, 2, C], f8)
            wvh_sb = persist.tile([P, KP, 2, CG], f8)
            wvl_sb = persist.tile([P, KP, 2, CG], f8)
            wp_sb = persist.tile([P, VB, C], bf16)
            # kqT[p, side(k/q), quad, s2, tok]
            kqT = persist.tile([P, 2, 2, 2, t], f8)
            v_aug = persist.tile([P, TT, HPC * (D + 1)], bf16)
            att = persist.tile([P, VB, t], bf16)

            # PE warm-up: dependency-free matmuls run during the input-DMA
            # window so the clock ramp is complete when real work starts.
            wu_a = persist.tile([P, P], bf16)
            wu_b = persist.tile([P, 512], bf16)
            nc.vector.memset(wu_a, 0.0)
            nc.vector.memset(wu_b, 0.0)
            for _ in range(10):
                wps = qpP.tile([P, 512], f32, name="wups", tag="qp", bufs=2)
                nc.tensor.matmul(wps, lhsT=wu_a, rhs=wu_b,
                                 start=True, stop=True,
                                 skip_group_check=True)

            # ---- loads, ordered for earliest compute start: wkq + x chunk 0
            # unblock the first kq units / S tiles; wv + xbf unblock v units
            # (first AV); wp is only needed by the first projection.
            xf8_r = xf8_d.ap().rearrange("(kp s p) t -> p kp s t", p=P, s=2)
            xf8l_r = xf8l_d.ap().rearrange(
                "(kp s p) t -> p kp s t", p=P, s=2)
            wkq_r = wkq_d.ap().rearrange("(kp s p) m -> p kp s m",
                                         p=P, s=2)
            # startup (quad 0, chunk 0) needs wkq blocks 0,1 (cols 0:256)
            # and 4,5 (cols 512:768) plus x chunk 0; v units for chunk c
            # need only the token-chunk-c slices of xf8/xf8l
            nc.sync.dma_start(wkq_sb[:, :, :, 0:512], wkq_r[:, :, :, 0:512])
            nc.sync.dma_start(xf8[:, :, :, 0:512], xf8_r[:, :, :, 0:512])
            nc.sync.dma_start(
                wvh_sb, wvh_d.ap().rearrange("(kp s p) m -> p kp s m",
                                             p=P, s=2))
            nc.sync.dma_start(
                wvl_sb, wvl_d.ap().rearrange("(kp s p) m -> p kp s m",
                                             p=P, s=2))
            nc.sync.dma_start(xf8l[:, :, :, 0:512], xf8l_r[:, :, :, 0:512])
            nc.sync.dma_start(wkq_sb[:, :, :, 512:1024],
                              wkq_r[:, :, :, 512:1024])
            nc.sync.dma_start(xf8[:, :, :, 512:t], xf8_r[:, :, :, 512:t])
            nc.sync.dma_start(xf8l[:, :, :, 512:t], xf8l_r[:, :, :, 512:t])
            nc.sync.dma_start(
                wp_sb, wp_d.ap().rearrange("(kt p) m -> p kt m", p=P))
            # only the ones-columns need initialising; v columns are written
            # by the v units
            nc.vector.memset(
                v_aug.rearrange("p tt (h e) -> p tt h e", e=D + 1)[:, :, :, D],
                1.0)

            # column position of block mb in the host-reordered wkq
            WKQ_POS = {0: 0, 1: 1, 4: 2, 5: 3, 2: 4, 3: 5, 6: 6, 7: 7}

            def emit_kq_unit(mb, c, on_act=False):
                # mb = (side, quad, s2) flat block index 0..7
                side, rem = divmod(mb, 4)
                quad, s2 = divmod(rem, 2)
                pos = WKQ_POS[mb]
                ps = qpP.tile([P, 512], f32, name="pskq", tag="qp", bufs=2)
                for kp in range(KP):
                    nc.tensor.matmul(
                        ps,
                        lhsT=wkq_sb[:, kp, :, pos * P:(pos + 1) * P],
                        rhs=xf8[:, kp, :, c * 512:(c + 1) * 512],
                        start=(kp == 0), stop=(kp == KP - 1),
                        perf_mode=DR, skip_group_check=True)
                out = kqT[:, side, quad, s2, c * 512:(c + 1) * 512]
                if on_act:
                    # startup copies run while ACT is otherwise idle,
                    # keeping the serial DVE copy chain off the first-S path
                    nc.scalar.copy(out, ps)
                else:
                    nc.vector.tensor_copy(out, ps)

            def emit_v_unit(tt):
                ps = qpP.tile([P, CG], f32, name="psv", tag="qp", bufs=2)
                terms = [(xf8, wvh_sb), (xf8l, wvh_sb), (xf8, wvl_sb)]
                for ti, (xs, ws) in enumerate(terms):
                    for kp in range(KP):
                        nc.tensor.matmul(
                            ps,
                            lhsT=xs[:, kp, :, tt * P:(tt + 1) * P],
                            rhs=ws[:, kp, :, :],
                            start=(ti == 0 and kp == 0),
                            stop=(ti == 2 and kp == KP - 1),
                            perf_mode=DR, skip_group_check=True)
                nc.vector.tensor_copy(
                    v_aug[:, tt, :].rearrange(
                        "p (h e) -> p h e", e=D + 1)[:, :, 0:D],
                    ps.rearrange("p (h d) -> p h d", d=D))

            def emit_proj_unit(mb, c):
                ps = qpP.tile([P, 512], f32, name="psp", tag="qp", bufs=2)
                for kt in range(VB):
                    nc.tensor.matmul(
                        ps,
                        lhsT=wp_sb[:, kt, mb * P:(mb + 1) * P],
                        rhs=att[:, kt, c * 512:(c + 1) * 512],
                        start=(kt == 0), stop=(kt == VB - 1),
                        skip_group_check=True)
                yt = yP.tile([P, 512], f32)
                if c == QC - 1:
                    # the last row's copies run in the kernel tail where the
                    # ACT engine is idle; keep them off the DVE queue
                    nc.scalar.copy(yt, ps)
                else:
                    nc.vector.tensor_copy(yt, ps)
                nc.sync.dma_start(
                    y_d[mb * P:(mb + 1) * P, c * 512:(c + 1) * 512], yt)

            def emit_s_phase(hp, c, fuse_av=False):
                """S + exp (+ causal mask) for chunk (hp, c).  Returns a
                closure emitting the AV/normalise/transpose phase, so the
                main loop can software-pipeline: S of chunk n+1 is emitted
                before AV of chunk n, keeping the ACT engine fed while the
                PE runs AV and filler units.  With fuse_av (final chunk),
                each AV qs-group is emitted right after its diagonal tile so
                the kernel tail shrinks; returns None."""
                quad = hp // 2
                lanes = (2 * (hp % 2), 2 * (hp % 2) + 1)
                heads = (2 * hp, 2 * hp + 1)

                def s_dr(out_ap, hi, j, w):
                    a = lanes[hi]
                    nc.tensor.matmul(
                        out_ap,
                        lhsT=kqT[32 * a:32 * a + 32, 0, quad, :,
                                 j * P:(j + 1) * P],
                        rhs=kqT[32 * a:32 * a + 32, 1, quad, :,
                                (c + 1) * 512 - w:(c + 1) * 512],
                        start=True, stop=True,
                        perf_mode=DR, skip_group_check=True,
                        tile_position=(32 * a, 0))

                # pt_slices[hi][j] -> AP covering q cols [off_j, 512) of the
                # exp'd S^T tile for (head hi, k-tile j), plus its offset
                pt_slices = [[None] * (4 * c + 4) for _ in range(2)]

                # full j-tiles, processed in (j, j+1) pairs per head
                for jp in range(2 * c):
                    for hi in range(2):
                        st = psS.tile([P, 2, 512], f32, name="st", tag="st")
                        for u in range(2):
                            s_dr(st[:, u, :], hi, 2 * jp + u, 512)
                        pt = ptP.tile([P, 2, 512], bf16, name="pt", tag="pt")
                        nc.scalar.activation(pt, st, Exp, scale=SCALE2)
                        for u in range(2):
                            pt_slices[hi][2 * jp + u] = (pt[:, u, :], 0)
                def emit_av_group(av, hi, qs):
                    h = heads[hi]
                    njs = 4 * c + qs + 1
                    for j in range(njs):
                        pap, off = pt_slices[hi][j]
                        lo = qs * P - off
                        nc.tensor.matmul(
                            av[:, qs, :],
                            lhsT=pap[:, lo:lo + P],
                            rhs=v_aug[:, j,
                                      h * (D + 1):(h + 1) * (D + 1)],
                            start=(j == 0), stop=(j == njs - 1),
                            skip_group_check=True)

                def norm_store(at_q, avs):
                    for hi in range(2):
                        rc = rcP.tile([P, 4], f32)
                        nc.vector.reciprocal(rc, avs[hi][:, :, D])
                        nc.vector.tensor_mul(
                            at_q[:, :, hi, :], avs[hi][:, :, 0:D],
                            rc[:, :, None].broadcast_to([P, 4, D]))
                    # blocked transpose: [128q, (qs hi d)] -> [128, qs, q]
                    nc.sync.dma_start(
                        att[:, hp, c * 512:(c + 1) * 512].rearrange(
                            "p (a b) -> p a b", b=P),
                        at_q, transpose=True)

                if fuse_av:
                    avs = [avP.tile([P, 4, D + 1], f32, name=f"av{hi}",
                                    tag="av") for hi in range(2)]

                # diagonal j-tiles: both heads share a tile per dj;
                # dj2 (w=256) and dj3 (w=128) are packed side by side in one
                # tile (384 f32 < one PSUM bank) sharing a single exp
                for dj, dj2 in ((0, None), (1, None), (2, 3)):
                    j = 4 * c + dj
                    w = 512 - P * dj
                    st = psS.tile([P, 2, 512], f32, name="std", tag="st")
                    for hi in range(2):
                        s_dr(st[:, hi, 0:w], hi, j, w)
                    wtot = w
                    if dj2 is not None:
                        w2 = 512 - P * dj2
                        for hi in range(2):
                            s_dr(st[:, hi, w:w + w2], hi, 4 * c + dj2, w2)
                        wtot = w + w2
                    pt = ptP.tile([P, 2, 512], bf16, name="ptd", tag="pt")
                    nc.scalar.activation(pt[:, :, 0:wtot], st[:, :, 0:wtot],
                                         Exp, scale=SCALE2)
                    nc.gpsimd.affine_select(
                        pt[:, :, 0:P], pt[:, :, 0:P],
                        pattern=[[0, 2], [1, P]],
                        compare_op=mybir.AluOpType.is_ge,
                        fill=0.0, base=0, channel_multiplier=-1)
                    for hi in range(2):
                        pt_slices[hi][j] = (pt[:, hi, 0:w], P * dj)
                    if dj2 is not None:
                        nc.gpsimd.affine_select(
                            pt[:, :, w:w + P], pt[:, :, w:w + P],
                            pattern=[[0, 2], [1, P]],
                            compare_op=mybir.AluOpType.is_ge,
                            fill=0.0, base=0, channel_multiplier=-1)
                        for hi in range(2):
                            pt_slices[hi][4 * c + dj2] = (
                                pt[:, hi, w:w + w2], P * dj2)
                    if fuse_av:
                        for hi in range(2):
                            emit_av_group(avs[hi], hi, dj)

                if fuse_av:
                    at_q = atP.tile([P, 4, 2, D], bf16)
                    norm_store(at_q, avs)
                    return None

                def av_phase():
                    at_q = atP.tile([P, 4, 2, D], bf16)
                    avs = []
                    for hi in range(2):
                        av = avP.tile([P, 4, D + 1], f32, name=f"av{hi}",
                                      tag="av")
                        avs.append(av)
                        for qs in range(4):
                            emit_av_group(av, hi, qs)
                    norm_store(at_q, avs)
                return av_phase

            # ---- startup: just enough for attn(hp=0, c=0); the v
            # units wait on the xf8l load, so they go in the fill queue
            # (needed only by the AVs, which the row-0 lag defers)
            for mb in (0, 1, 4, 5):        # (k, quad0, s2=0/1), (q, quad0, ...)
                emit_kq_unit(mb, 0, on_act=True)

            # Remaining kq/v units, emitted as PE filler between attention
            # chunks.  Tile discovers dependencies from TRACE order, so a
            # producer MUST be emitted before its first consumer chunk; each
            # fill carries the global chunk index (ci = 4*c + hp) it is first
            # needed by.
            def cdiv(a, b):
                return -(-a // b)

            fills = []
            for tt in range(4):
                fills.append(((1, 0), ("v", tt)))
            for tt in range(4, TT):
                # attn(*, c) AV reads v tiles tt <= 4c+3 (exact need: the
                # early v units would stall the PE on the xf8l/wv loads)
                fills.append(
                    ((4 * max(0, cdiv(tt - 3, 4)), 0), ("v", tt)))
            for side in range(2):
                for quad in range(2):
                    for s2 in range(2):
                        mb = side * 4 + quad * 2 + s2
                        for cc in range(QC):
                            if mb in (0, 1, 4, 5) and cc == 0:
                                continue
                            # k side chunk cc needed by attn(2*quad, c>=cc);
                            # q side chunk cc needed by attn(2*quad, cc)
                            fills.append(
                                ((4 * cc + 2 * quad, 1), ("kq", mb, cc)))
            fills.sort(key=lambda f: f[0])

            nchunks = QC * 4
            emitted = 0

            def emit_fills(upto):
                nonlocal emitted
                while emitted < min(upto, len(fills)):
                    _, f = fills[emitted]
                    if f[0] == "kq":
                        emit_kq_unit(f[1], f[2])
                    else:
                        emit_v_unit(f[1])
                    emitted += 1

            # Software-pipelined schedule: S/exp of chunk n+1 is emitted
            # before the AV of chunk n (row 0 keeps all four S-phases ahead
            # so ACT covers the v-unit load window); projection of row c is
            # spread across the AV slots of row c+1.
            pending_av = []      # FIFO of av_phase closures
            pending_proj = []    # FIFO of (mb, c) projection units
            for c in range(QC):
                for hp in range(4):
                    ci = 4 * c + hp
                    # everything this chunk reads must already be emitted
                    while emitted < len(fills) and fills[emitted][0][0] <= ci:
                        emit_fills(emitted + 1)
                    pending_av.append(emit_s_phase(hp, c))
                    # lag taper: row 0 keeps all four S-phases ahead of the
                    # first AV (covers the v-unit load window), row 1 drains
                    # the backlog gradually, steady state keeps one chunk of
                    # S/exp in flight ahead of AV
                    lag = (4 if c == 0 else
                           max(1, 3 - hp) if c == 1 else 1)
                    while len(pending_av) > lag:
                        pending_av.pop(0)()
                        # projection of row cc is paced two rows behind (the
                        # ACT-slack rows); the last row takes double rate
                        nproj = 4 if c == QC - 1 else 2
                        for _ in range(nproj):
                            if pending_proj and (
                                    pending_proj[0][1] <= c - 2
                                    or c == QC - 1):
                                mb, cc = pending_proj.pop(0)
                                emit_proj_unit(mb, cc)
                    emit_fills(((ci + 3) * len(fills)) // nchunks)
                pending_proj.extend((mb, c) for mb in range(C // P))
            emit_fills(len(fills))
            while pending_av:
                pending_av.pop(0)()
            for mb, cc in pending_proj:
                emit_proj_unit(mb, cc)

    nc.compile()
    return nc


def _get_compiled(t=T):
    if t not in _compiled:
        _compiled[t] = _build(t)
    return _compiled[t]


def make_in_maps(x, W_qkv, W_proj):
    bf = ml_dtypes.bfloat16
    f8 = ml_dtypes.float8_e4m3
    x = np.asarray(x, dtype=np.float32)
    W_qkv = np.asarray(W_qkv, dtype=np.float32)
    W_proj = np.asarray(W_proj, dtype=np.float32)

    in_maps = []
    for core in range(8):
        b, g = core // 2, core % 2
        xT = np.ascontiguousarray(x[b].T)          # [C, T]
        # wkq column order: block (side, quad, s2): lane-major 32-channel
        # slices of heads 8g+4*quad..+3, d-range [32*s2, 32*s2+32)
        cols = []
        for side, quad in ((0, 0), (1, 0), (0, 1), (1, 1)):
            base = side * C
            for s2 in range(2):
                for lq in range(4):
                    h = 8 * g + 4 * quad + lq
                    st = base + h * D + s2 * 32
                    cols.append(np.arange(st, st + 32))
        cols = np.concatenate(cols)
        xf8 = xT.astype(f8)
        wvs = np.ascontiguousarray(
            W_qkv[:, 2 * C + g * CG:2 * C + (g + 1) * CG]) * WS
        wvh = wvs.astype(f8)
        in_maps.append({
            "xf8": xf8,
            "xf8l": (xT - xf8.astype(np.float32)).astype(f8),
            "wkq": (W_qkv[:, cols] * WS).astype(f8),
            "wvh": wvh,
            "wvl": (wvs - wvh.astype(np.float32)).astype(f8),
            # v (and hence att) carries the extra WS factor; fold the
            # inverse into the projection weights
            "wp": np.ascontiguousarray(
                W_proj[g * CG:(g + 1) * CG, :] / WS).astype(bf),
        })
    return in_maps


def _run_axon_nodonate(nc, in_maps, n_cores=8):
    """Execute via PJRT/shard_map WITHOUT output-buffer donation.

    bass2jax.run_bass_via_pjrt donates the zero output operands; under the
    axon transport that donation intermittently corrupts multi-core results.
    This kernel writes every element of its output, so donation is not
    needed for correctness -- pass non-donated zero operands instead.
    """
    import jax
    from jax.sharding import Mesh, PartitionSpec
    from jax.experimental.shard_map import shard_map
    import concourse.mybir as mybir
    from concourse.bass2jax import _bass_exec_p, install_neuronx_cc_hook

    install_neuronx_cc_hook()
    in_names, out_names, out_avals = [], [], []
    for alloc in nc.m.functions[0].allocations:
        if not isinstance(alloc, mybir.MemoryLocationSet):
            continue
        name = alloc.memorylocations[0].name
        if alloc.kind == "ExternalInput":
            in_names.append(name)
        elif alloc.kind == "ExternalOutput":
            out_names.append(name)
            out_avals.append(jax.core.ShapedArray(
                tuple(alloc.tensor_shape), mybir.dt.np(alloc.dtype)))
    n_params = len(in_names)
    all_names = in_names + out_names
    pid_name = nc.partition_id_tensor.name if nc.partition_id_tensor else None

    def _body(*args):
        return tuple(_bass_exec_p.bind(
            *args,
            out_avals=tuple(out_avals),
            in_names=tuple(all_names),
            out_names=tuple(out_names),
            lowering_input_output_aliases=(),
            sim_require_finite=True,
            sim_require_nnan=True,
            nc=nc,
        ))

    devices = jax.devices()[:n_cores]
    mesh = Mesh(np.asarray(devices), ("core",))
    fn = jax.jit(
        shard_map(_body, mesh=mesh,
                  in_specs=(PartitionSpec("core"),) * (n_params + len(out_names)),
                  out_specs=(PartitionSpec("core"),) * len(out_names),
                  check_rep=False),
        keep_unused=True)
    concat_in = [
        np.concatenate([
            np.asarray(in_maps[c].get(
                nm, np.array([[c]], dtype=np.uint32) if nm == pid_name
                else None))
            for c in range(n_cores)], 0)
        for nm in in_names
    ]
    concat_zeros = [
        np.zeros((n_cores * a.shape[0], *a.shape[1:]), a.dtype)
        for a in out_avals
    ]
    out = fn(*concat_in, *concat_zeros)
    return [
        {nm: np.asarray(out[i]).reshape(n_cores, *out_avals[i].shape)[c]
         for i, nm in enumerate(out_names)}
        for c in range(n_cores)
    ]


def kernel(x, W_qkv, W_proj, _trace=False):
    from concourse._compat import axon_active

    nc = _get_compiled()
    in_maps = make_in_maps(x, W_qkv, W_proj)
    if axon_active():
        results = _run_axon_nodonate(nc, in_maps)
    else:
        import concourse.bass_utils as bass_utils
        res = bass_utils.run_bass_kernel_spmd(
            nc, in_maps, core_ids=list(range(8)), trace=_trace)
        if _trace:
            kernel.last_results = res
        results = res.results
    y = np.zeros((B, T, C), np.float32)
    for core in range(8):
        y[core // 2] += results[core]["y"].T
    return y


# revision 41
# speedup vs baseline: 1.4282x; 1.0028x over previous
"""Causal self-attention Trainium2 kernel (fp8 DoubleRow + AV-swap design).

Problem: y = CausalSelfAttention(x) with B=4, T=2048, C=1024, H=16 heads,
head_dim D=64, qkv split order (k, q, v), softmax scale C**-0.5.

Sharding (8 cores): core = 2*b + g  -> batch b in 0..3, head-group g in 0..1
(8 local heads per core).  Each core computes qkv for its 8 heads, causal
attention, and the partial projection y_partial = att_out @ W_proj[g rows].
The host sums the two partial projections per batch.

Key device-side structure (per core):
  kq:   fp8e4 DoubleRow matmuls (2 k-slices per instruction, 0.5 cyc/row).
        W_qkv columns are host-reordered so PSUM partitions land directly in
        the S-ready layout: block (side, quad, s2) holds d-channels
        [s2*32, s2*32+32) of heads 4*quad..4*quad+3 (lane-major).  W scaled
        by 32 on host so fp8 stays in normal range; exp scale divides by
        32*32.
  kqT:  [128, side, quad, s2, T] fp8 - head h lives on partitions
        32*(h%4)..+32 of quad h//4, with head-dim split across s2 in {0,1}.
  S:    per (head, j-tile) one fp8 DoubleRow matmul: lhsT [32, 2, 128] (k),
        rhs [32, 2, 512] (q chunk) -> S^T [128k, 512q] in PSUM (256 cyc).
  exp:  ACT, scale = C**-0.5/1024, bf16 out (pt tiles).  Full j-tiles
        batched in pairs; diagonal tiles column-sliced to the valid width
        and masked with gpsimd affine_select (leading 128 cols).
  AV:   transposed accumulation: out[q=128, 65] += pt_j[:, qslice]^T(lhsT)
        @ v_aug_j[128, 65](rhs, moving bf16) -> 65 cyc per instruction.
        Column 64 (ones in v_aug) accumulates the softmax denominator into
        the same partition as its q row.
  norm: DVE reciprocal [128, 4] + one broadcast tensor_mul per (pair, chunk,
        head) -> att_q [128q, qs, hi, 64] bf16.
  att:  one blocked DMA transpose per (pair, chunk): [128, 4*128] ->
        [128, 4, 128] producing channel-major att for the projection.
  proj: y^T[cout 128, q 512] = wp(lhsT) @ att(rhs, bf16) per (mb, chunk),
        emitted after each attention chunk-row completes (chunk-major loop)
        so projection overlaps the attention tail.

Scheduling: chunk-major (c outer, head-pair inner); kq/v units beyond the
startup set are emitted as PE filler between attention chunks (ACT is the
bottleneck engine; PE has slack).
"""

import numpy as np
import ml_dtypes

B, T, C, H = 4, 2048, 1024, 16  # noqa
D = C // H          # 64
HPC = H // 2        # 8 heads per core
CG = C // 2         # 512 channels per head group
P = 128
WS = 32.0           # host-side W_qkv scale for fp8 range
SCALE = float(C) ** -0.5

_compiled = {}


def _build(t=T):
    import concourse.bacc as bacc
    import concourse.tile as tile
    import concourse.mybir as mybir

    f32 = mybir.dt.float32
    bf16 = mybir.dt.bfloat16
    f8 = mybir.dt.float8e4
    DR = mybir.MatmulPerfMode.DoubleRow
    Exp = mybir.ActivationFunctionType.Exp

    KT = C // P            # 8 contraction tiles over C
    KP = KT // 2           # 4 DoubleRow contraction pairs
    TT = t // P            # token tiles of 128
    QC = t // 512          # q chunks of 512
    VB = CG // P           # 4 att channel blocks (= head pairs)
    SCALE2 = SCALE / (WS * WS)

    nc = bacc.Bacc("TRN2", target_bir_lowering=False, debug=False,
                   num_devices=8)

    xf8_d = nc.dram_tensor("xf8", [C, t], f8, kind="ExternalInput")
    xf8l_d = nc.dram_tensor("xf8l", [C, t], f8, kind="ExternalInput")
    wkq_d = nc.dram_tensor("wkq", [C, C], f8, kind="ExternalInput")
    wvh_d = nc.dram_tensor("wvh", [C, CG], f8, kind="ExternalInput")
    wvl_d = nc.dram_tensor("wvl", [C, CG], f8, kind="ExternalInput")
    wp_d = nc.dram_tensor("wp", [CG, C], bf16, kind="ExternalInput")
    y_d = nc.dram_tensor("y", [C, t], f32, kind="ExternalOutput")

    with tile.TileContext(nc) as tc:
        with (
            tc.tile_pool(name="persist", bufs=1) as persist,
            tc.tile_pool(name="psS", bufs=2, space="PSUM") as psS,
            tc.tile_pool(name="avP", bufs=2, space="PSUM") as avP,
            tc.tile_pool(name="qpP", bufs=2, space="PSUM") as qpP,
            tc.tile_pool(name="ptP", bufs=26) as ptP,
            tc.tile_pool(name="rcP", bufs=4) as rcP,
            tc.tile_pool(name="atP", bufs=3) as atP,
            tc.tile_pool(name="yP", bufs=3) as yP,
        ):
            xf8 = persist.tile([P, KP, 2, t], f8)
            xf8l = persist.tile([P, KP, 2, t], f8)
            wkq_sb = persist.tile([P, KP, 2, C], f8)
            wvh_sb = persist.tile([P, KP, 2, CG], f8)
            wvl_sb = persist.tile([P, KP, 2, CG], f8)
            wp_sb = persist.tile([P, VB, C], bf16)
            # kqT[p, side(k/q), quad, s2, tok]
            kqT = persist.tile([P, 2, 2, 2, t], f8)
            v_aug = persist.tile([P, TT, HPC * (D + 1)], bf16)
            att = persist.tile([P, VB, t], bf16)

            # PE warm-up: dependency-free matmuls run during the input-DMA
            # window so the clock ramp is complete when real work starts.
            wu_a = persist.tile([P, P], bf16)
            wu_b = persist.tile([P, 512], bf16)
            nc.vector.memset(wu_a, 0.0)
            nc.vector.memset(wu_b, 0.0)
            for _ in range(14):
                wps = qpP.tile([P, 512], f32, name="wups", tag="qp", bufs=2)
                nc.tensor.matmul(wps, lhsT=wu_a, rhs=wu_b,
                                 start=True, stop=True,
                                 skip_group_check=True)

            # ---- loads, ordered for earliest compute start: wkq + x chunk 0
            # unblock the first kq units / S tiles; wv + xf8l unblock v units
            # (first AV); wp is only needed by the first projection.
            xf8_r = xf8_d.ap().rearrange("(kp s p) t -> p kp s t", p=P, s=2)
            xf8l_r = xf8l_d.ap().rearrange(
                "(kp s p) t -> p kp s t", p=P, s=2)
            wkq_r = wkq_d.ap().rearrange("(kp s p) m -> p kp s m",
                                         p=P, s=2)
            # startup (quad 0, chunk 0) needs wkq blocks 0,1 (cols 0:256)
            # and 4,5 (cols 512:768) plus x chunk 0; v units for chunk c
            # need only the token-chunk-c slices of xf8/xf8l
            nc.sync.dma_start(wkq_sb[:, :, :, 0:512], wkq_r[:, :, :, 0:512])
            nc.sync.dma_start(xf8[:, :, :, 0:512], xf8_r[:, :, :, 0:512])
            nc.sync.dma_start(
                wvh_sb, wvh_d.ap().rearrange("(kp s p) m -> p kp s m",
                                             p=P, s=2))
            nc.sync.dma_start(
                wvl_sb, wvl_d.ap().rearrange("(kp s p) m -> p kp s m",
                                             p=P, s=2))
            nc.sync.dma_start(xf8l[:, :, :, 0:512], xf8l_r[:, :, :, 0:512])
            nc.sync.dma_start(wkq_sb[:, :, :, 512:1024],
                              wkq_r[:, :, :, 512:1024])
            nc.sync.dma_start(xf8[:, :, :, 512:t], xf8_r[:, :, :, 512:t])
            nc.sync.dma_start(xf8l[:, :, :, 512:t], xf8l_r[:, :, :, 512:t])
            nc.sync.dma_start(
                wp_sb, wp_d.ap().rearrange("(kt p) m -> p kt m", p=P))
            # only the ones-columns need initialising; v columns are written
            # by the v units
            nc.vector.memset(
                v_aug.rearrange("p tt (h e) -> p tt h e", e=D + 1)[:, :, :, D],
                1.0)

            # column position of block mb in the host-reordered wkq
            WKQ_POS = {0: 0, 1: 1, 4: 2, 5: 3, 2: 4, 3: 5, 6: 6, 7: 7}

            def emit_kq_unit(mb, c, on_act=False):
                # mb = (side, quad, s2) flat block index 0..7
                side, rem = divmod(mb, 4)
                quad, s2 = divmod(rem, 2)
                pos = WKQ_POS[mb]
                ps = qpP.tile([P, 512], f32, name="pskq", tag="qp", bufs=2)
                for kp in range(KP):
                    nc.tensor.matmul(
                        ps,
                        lhsT=wkq_sb[:, kp, :, pos * P:(pos + 1) * P],
                        rhs=xf8[:, kp, :, c * 512:(c + 1) * 512],
                        start=(kp == 0), stop=(kp == KP - 1),
                        perf_mode=DR, skip_group_check=True)
                out = kqT[:, side, quad, s2, c * 512:(c + 1) * 512]
                if on_act:
                    # startup copies run while ACT is otherwise idle,
                    # keeping the serial DVE copy chain off the first-S path
                    nc.scalar.copy(out, ps)
                else:
                    nc.vector.tensor_copy(out, ps)

            def emit_v_unit(tt):
                ps = qpP.tile([P, CG], f32, name="psv", tag="qp", bufs=2)
                terms = [(xf8, wvh_sb), (xf8l, wvh_sb), (xf8, wvl_sb)]
                for ti, (xs, ws) in enumerate(terms):
                    for kp in range(KP):
                        nc.tensor.matmul(
                            ps,
                            lhsT=xs[:, kp, :, tt * P:(tt + 1) * P],
                            rhs=ws[:, kp, :, :],
                            start=(ti == 0 and kp == 0),
                            stop=(ti == 2 and kp == KP - 1),
                            perf_mode=DR, skip_group_check=True)
                nc.vector.tensor_copy(
                    v_aug[:, tt, :].rearrange(
                        "p (h e) -> p h e", e=D + 1)[:, :, 0:D],
                    ps.rearrange("p (h d) -> p h d", d=D))

            def emit_proj_unit(mb, c):
                ps = qpP.tile([P, 512], f32, name="psp", tag="qp", bufs=2)
                for kt in range(VB):
                    nc.tensor.matmul(
                        ps,
                        lhsT=wp_sb[:, kt, mb * P:(mb + 1) * P],
                        rhs=att[:, kt, c * 512:(c + 1) * 512],
                        start=(kt == 0), stop=(kt == VB - 1),
                        skip_group_check=True)
                yt = yP.tile([P, 512], f32)
                if c == QC - 1:
                    # the last row's copies run in the kernel tail where the
                    # ACT engine is idle; keep them off the DVE queue
                    nc.scalar.copy(yt, ps)
                else:
                    nc.vector.tensor_copy(yt, ps)
                nc.sync.dma_start(
                    y_d[mb * P:(mb + 1) * P, c * 512:(c + 1) * 512], yt)

            def emit_s_phase(hp, c, fuse_av=False):
                """S + exp (+ causal mask) for chunk (hp, c).  Returns a
                closure emitting the AV/normalise/transpose phase, so the
                main loop can software-pipeline: S of chunk n+1 is emitted
                before AV of chunk n, keeping the ACT engine fed while the
                PE runs AV and filler units.  With fuse_av (final chunk),
                each AV qs-group is emitted right after its diagonal tile so
                the kernel tail shrinks; returns None."""
                quad = hp // 2
                lanes = (2 * (hp % 2), 2 * (hp % 2) + 1)
                heads = (2 * hp, 2 * hp + 1)

                def s_dr(out_ap, hi, j, w):
                    a = lanes[hi]
                    nc.tensor.matmul(
                        out_ap,
                        lhsT=kqT[32 * a:32 * a + 32, 0, quad, :,
                                 j * P:(j + 1) * P],
                        rhs=kqT[32 * a:32 * a + 32, 1, quad, :,
                                (c + 1) * 512 - w:(c + 1) * 512],
                        start=True, stop=True,
                        perf_mode=DR, skip_group_check=True,
                        tile_position=(32 * a, 0))

                # pt_slices[hi][j] -> AP covering q cols [off_j, 512) of the
                # exp'd S^T tile for (head hi, k-tile j), plus its offset
                pt_slices = [[None] * (4 * c + 4) for _ in range(2)]

                # full j-tiles, processed in (j, j+1) pairs per head
                for jp in range(2 * c):
                    for hi in range(2):
                        st = psS.tile([P, 2, 512], f32, name="st", tag="st")
                        for u in range(2):
                            s_dr(st[:, u, :], hi, 2 * jp + u, 512)
                        pt = ptP.tile([P, 2, 512], bf16, name="pt", tag="pt")
                        nc.scalar.activation(pt, st, Exp, scale=SCALE2)
                        for u in range(2):
                            pt_slices[hi][2 * jp + u] = (pt[:, u, :], 0)
                def emit_av_group(av, hi, qs):
                    h = heads[hi]
                    njs = 4 * c + qs + 1
                    for j in range(njs):
                        pap, off = pt_slices[hi][j]
                        lo = qs * P - off
                        nc.tensor.matmul(
                            av[:, qs, :],
                            lhsT=pap[:, lo:lo + P],
                            rhs=v_aug[:, j,
                                      h * (D + 1):(h + 1) * (D + 1)],
                            start=(j == 0), stop=(j == njs - 1),
                            skip_group_check=True)

                def norm_store(at_q, avs):
                    for hi in range(2):
                        rc = rcP.tile([P, 4], f32)
                        nc.vector.reciprocal(rc, avs[hi][:, :, D])
                        nc.vector.tensor_mul(
                            at_q[:, :, hi, :], avs[hi][:, :, 0:D],
                            rc[:, :, None].broadcast_to([P, 4, D]))
                    # blocked transpose: [128q, (qs hi d)] -> [128, qs, q]
                    nc.sync.dma_start(
                        att[:, hp, c * 512:(c + 1) * 512].rearrange(
                            "p (a b) -> p a b", b=P),
                        at_q, transpose=True)

                if fuse_av:
                    avs = [avP.tile([P, 4, D + 1], f32, name=f"av{hi}",
                                    tag="av") for hi in range(2)]

                # diagonal j-tiles: both heads share a tile per dj;
                # dj2 (w=256) and dj3 (w=128) are packed side by side in one
                # tile (384 f32 < one PSUM bank) sharing a single exp
                for dj, dj2 in ((0, None), (1, None), (2, 3)):
                    j = 4 * c + dj
                    w = 512 - P * dj
                    st = psS.tile([P, 2, 512], f32, name="std", tag="st")
                    for hi in range(2):
                        s_dr(st[:, hi, 0:w], hi, j, w)
                    wtot = w
                    if dj2 is not None:
                        w2 = 512 - P * dj2
                        for hi in range(2):
                            s_dr(st[:, hi, w:w + w2], hi, 4 * c + dj2, w2)
                        wtot = w + w2
                    pt = ptP.tile([P, 2, 512], bf16, name="ptd", tag="pt")
                    nc.scalar.activation(pt[:, :, 0:wtot], st[:, :, 0:wtot],
                                         Exp, scale=SCALE2)
                    nc.gpsimd.affine_select(
                        pt[:, :, 0:P], pt[:, :, 0:P],
                        pattern=[[0, 2], [1, P]],
                        compare_op=mybir.AluOpType.is_ge,
                        fill=0.0, base=0, channel_multiplier=-1)
                    for hi in range(2):
                        pt_slices[hi][j] = (pt[:, hi, 0:w], P * dj)
                    if dj2 is not None:
                        nc.gpsimd.affine_select(
                            pt[:, :, w:w + P], pt[:, :, w:w + P],
                            pattern=[[0, 2], [1, P]],
                            compare_op=mybir.AluOpType.is_ge,
                            fill=0.0, base=0, channel_multiplier=-1)
                        for hi in range(2):
                            pt_slices[hi][4 * c + dj2] = (
                                pt[:, hi, w:w + w2], P * dj2)
                    if fuse_av:
                        for hi in range(2):
                            emit_av_group(avs[hi], hi, dj)

                if fuse_av:
                    at_q = atP.tile([P, 4, 2, D], bf16)
                    norm_store(at_q, avs)
                    return None

                def av_phase():
                    at_q = atP.tile([P, 4, 2, D], bf16)
                    avs = []
                    for hi in range(2):
                        av = avP.tile([P, 4, D + 1], f32, name=f"av{hi}",
                                      tag="av")
                        avs.append(av)
                        for qs in range(4):
                            emit_av_group(av, hi, qs)
                    norm_store(at_q, avs)
                return av_phase

            # ---- startup: just enough for attn(hp=0, c=0); the v
            # units wait on the xf8l load, so they go in the fill queue
            # (needed only by the AVs, which the row-0 lag defers)
            for mb in (0, 1, 4, 5):        # (k, quad0, s2=0/1), (q, quad0, ...)
                emit_kq_unit(mb, 0, on_act=True)
            for tt in range(4):
                emit_v_unit(tt)

            # Remaining kq/v units, emitted as PE filler between attention
            # chunks.  Tile discovers dependencies from TRACE order, so a
            # producer MUST be emitted before its first consumer chunk; each
            # fill carries the global chunk index (ci = 4*c + hp) it is first
            # needed by.
            def cdiv(a, b):
                return -(-a // b)

            fills = []
            for tt in range(4, TT):
                # attn(*, c) AV reads v tiles tt <= 4c+3 (exact need: the
                # early v units would stall the PE on the xf8l/wv loads)
                fills.append(
                    ((4 * max(0, cdiv(tt - 3, 4)), 0), ("v", tt)))
            for side in range(2):
                for quad in range(2):
                    for s2 in range(2):
                        mb = side * 4 + quad * 2 + s2
                        for cc in range(QC):
                            if mb in (0, 1, 4, 5) and cc == 0:
                                continue
                            # k side chunk cc needed by attn(2*quad, c>=cc);
                            # q side chunk cc needed by attn(2*quad, cc)
                            fills.append(
                                ((4 * cc + 2 * quad, 1), ("kq", mb, cc)))
            fills.sort(key=lambda f: f[0])

            nchunks = QC * 4
            emitted = 0

            def emit_fills(upto):
                nonlocal emitted
                while emitted < min(upto, len(fills)):
                    _, f = fills[emitted]
                    if f[0] == "kq":
                        emit_kq_unit(f[1], f[2])
                    else:
                        emit_v_unit(f[1])
                    emitted += 1

            # Software-pipelined schedule: S/exp of chunk n+1 is emitted
            # before the AV of chunk n (row 0 keeps all four S-phases ahead
            # so ACT covers the v-unit load window); projection of row c is
            # spread across the AV slots of row c+1.
            pending_av = []      # FIFO of av_phase closures
            pending_proj = []    # FIFO of (mb, c) projection units
            for c in range(QC):
                for hp in range(4):
                    ci = 4 * c + hp
                    # everything this chunk reads must already be emitted
                    while emitted < len(fills) and fills[emitted][0][0] <= ci:
                        emit_fills(emitted + 1)
                    pending_av.append(emit_s_phase(hp, c))
                    # lag taper: row 0 keeps all four S-phases ahead of the
                    # first AV (covers the v-unit load window), row 1 drains
                    # the backlog gradually, steady state keeps one chunk of
                    # S/exp in flight ahead of AV
                    lag = (4 if c == 0 else
                           max(1, 4 - hp) if c == 1 else 1)
                    while len(pending_av) > lag:
                        pending_av.pop(0)()
                        # projection of row cc is paced two rows behind (the
                        # ACT-slack rows); the last row takes double rate
                        nproj = 4 if c == QC - 1 else 2
                        for _ in range(nproj):
                            if pending_proj and (
                                    pending_proj[0][1] <= c - 2
                                    or c == QC - 1):
                                mb, cc = pending_proj.pop(0)
                                emit_proj_unit(mb, cc)
                    emit_fills(((ci + 3) * len(fills)) // nchunks)
                pending_proj.extend((mb, c) for mb in range(C // P))
            emit_fills(len(fills))
            while pending_av:
                pending_av.pop(0)()
            for mb, cc in pending_proj:
                emit_proj_unit(mb, cc)

    nc.compile()
    return nc


def _get_compiled(t=T):
    if t not in _compiled:
        _compiled[t] = _build(t)
    return _compiled[t]


def make_in_maps(x, W_qkv, W_proj):
    bf = ml_dtypes.bfloat16
    f8 = ml_dtypes.float8_e4m3
    x = np.asarray(x, dtype=np.float32)
    W_qkv = np.asarray(W_qkv, dtype=np.float32)
    W_proj = np.asarray(W_proj, dtype=np.float32)

    in_maps = []
    for core in range(8):
        b, g = core // 2, core % 2
        xT = np.ascontiguousarray(x[b].T)          # [C, T]
        # wkq column order: block (side, quad, s2): lane-major 32-channel
        # slices of heads 8g+4*quad..+3, d-range [32*s2, 32*s2+32)
        cols = []
        for side, quad in ((0, 0), (1, 0), (0, 1), (1, 1)):
            base = side * C
            for s2 in range(2):
                for lq in range(4):
                    h = 8 * g + 4 * quad + lq
                    st = base + h * D + s2 * 32
                    cols.append(np.arange(st, st + 32))
        cols = np.concatenate(cols)
        xf8 = xT.astype(f8)
        wvs = np.ascontiguousarray(
            W_qkv[:, 2 * C + g * CG:2 * C + (g + 1) * CG]) * WS
        wvh = wvs.astype(f8)
        in_maps.append({
            "xf8": xf8,
            "xf8l": (xT - xf8.astype(np.float32)).astype(f8),
            "wkq": (W_qkv[:, cols] * WS).astype(f8),
            "wvh": wvh,
            "wvl": (wvs - wvh.astype(np.float32)).astype(f8),
            # v (and hence att) carries the extra WS factor; fold the
            # inverse into the projection weights
            "wp": np.ascontiguousarray(
                W_proj[g * CG:(g + 1) * CG, :] / WS).astype(bf),
        })
    return in_maps


def _run_axon_nodonate(nc, in_maps, n_cores=8):
    """Execute via PJRT/shard_map WITHOUT output-buffer donation.

    bass2jax.run_bass_via_pjrt donates the zero output operands; under the
    axon transport that donation intermittently corrupts multi-core results.
    This kernel writes every element of its output, so donation is not
    needed for correctness -- pass non-donated zero operands instead.
    """
    import jax
    from jax.sharding import Mesh, PartitionSpec
    from jax.experimental.shard_map import shard_map
    import concourse.mybir as mybir
    from concourse.bass2jax import _bass_exec_p, install_neuronx_cc_hook

    install_neuronx_cc_hook()
    in_names, out_names, out_avals = [], [], []
    for alloc in nc.m.functions[0].allocations:
        if not isinstance(alloc, mybir.MemoryLocationSet):
            continue
        name = alloc.memorylocations[0].name
        if alloc.kind == "ExternalInput":
            in_names.append(name)
        elif alloc.kind == "ExternalOutput":
            out_names.append(name)
            out_avals.append(jax.core.ShapedArray(
                tuple(alloc.tensor_shape), mybir.dt.np(alloc.dtype)))
    n_params = len(in_names)
    all_names = in_names + out_names
    pid_name = nc.partition_id_tensor.name if nc.partition_id_tensor else None

    def _body(*args):
        return tuple(_bass_exec_p.bind(
            *args,
            out_avals=tuple(out_avals),
            in_names=tuple(all_names),
            out_names=tuple(out_names),
            lowering_input_output_aliases=(),
            sim_require_finite=True,
            sim_require_nnan=True,
            nc=nc,
        ))

    devices = jax.devices()[:n_cores]
    mesh = Mesh(np.asarray(devices), ("core",))
    fn = jax.jit(
        shard_map(_body, mesh=mesh,
                  in_specs=(PartitionSpec("core"),) * (n_params + len(out_names)),
                  out_specs=(PartitionSpec("core"),) * len(out_names),
                  check_rep=False),
        keep_unused=True)
    concat_in = [
        np.concatenate([
            np.asarray(in_maps[c].get(
                nm, np.array([[c]], dtype=np.uint32) if nm == pid_name
                else None))
            for c in range(n_cores)], 0)
        for nm in in_names
    ]
    concat_zeros = [
        np.zeros((n_cores * a.shape[0], *a.shape[1:]), a.dtype)
        for a in out_avals
    ]
    out = fn(*concat_in, *concat_zeros)
    return [
        {nm: np.asarray(out[i]).reshape(n_cores, *out_avals[i].shape)[c]
         for i, nm in enumerate(out_names)}
        for c in range(n_cores)
    ]


def kernel(x, W_qkv, W_proj, _trace=False):
    from concourse._compat import axon_active

    nc = _get_compiled()
    in_maps = make_in_maps(x, W_qkv, W_proj)
    if axon_active():
        results = _run_axon_nodonate(nc, in_maps)
    else:
        import concourse.bass_utils as bass_utils
        res = bass_utils.run_bass_kernel_spmd(
            nc, in_maps, core_ids=list(range(8)), trace=_trace)
        if _trace:
            kernel.last_results = res
        results = res.results
    y = np.zeros((B, T, C), np.float32)
    for core in range(8):
        y[core // 2] += results[core]["y"].T
    return y


# revision 48
# speedup vs baseline: 1.4464x; 1.0128x over previous
"""Causal self-attention Trainium2 kernel (fp8 DoubleRow + AV-swap design).

Problem: y = CausalSelfAttention(x) with B=4, T=2048, C=1024, H=16 heads,
head_dim D=64, qkv split order (k, q, v), softmax scale C**-0.5.

Sharding (8 cores): core = 2*b + g  -> batch b in 0..3, head-group g in 0..1
(8 local heads per core).  Each core computes qkv for its 8 heads, causal
attention, and the partial projection y_partial = att_out @ W_proj[g rows].
The host sums the two partial projections per batch.

Key device-side structure (per core):
  kq:   fp8e4 DoubleRow matmuls (2 k-slices per instruction, 0.5 cyc/row).
        W_qkv columns are host-reordered so PSUM partitions land directly in
        the S-ready layout: block (side, quad, s2) holds d-channels
        [s2*32, s2*32+32) of heads 4*quad..4*quad+3 (lane-major).  W scaled
        by 32 on host so fp8 stays in normal range; exp scale divides by
        32*32.
  kqT:  [128, side, quad, s2, T] fp8 - head h lives on partitions
        32*(h%4)..+32 of quad h//4, with head-dim split across s2 in {0,1}.
  S:    per (head, j-tile) one fp8 DoubleRow matmul: lhsT [32, 2, 128] (k),
        rhs [32, 2, 512] (q chunk) -> S^T [128k, 512q] in PSUM (256 cyc).
  exp:  ACT, scale = C**-0.5/1024, bf16 out (pt tiles).  Full j-tiles
        batched in pairs; diagonal tiles column-sliced to the valid width
        and masked with gpsimd affine_select (leading 128 cols).
  AV:   transposed accumulation: out[q=128, 65] += pt_j[:, qslice]^T(lhsT)
        @ v_aug_j[128, 65](rhs, moving bf16) -> 65 cyc per instruction.
        Column 64 (ones in v_aug) accumulates the softmax denominator into
        the same partition as its q row.
  norm: DVE reciprocal [128, 4] + one broadcast tensor_mul per (pair, chunk,
        head) -> att_q [128q, qs, hi, 64] bf16.
  att:  one blocked DMA transpose per (pair, chunk): [128, 4*128] ->
        [128, 4, 128] producing channel-major att for the projection.
  proj: y^T[cout 128, q 512] = wp(lhsT) @ att(rhs, bf16) per (mb, chunk),
        emitted after each attention chunk-row completes (chunk-major loop)
        so projection overlaps the attention tail.

Scheduling: chunk-major (c outer, head-pair inner); kq/v units beyond the
startup set are emitted as PE filler between attention chunks (ACT is the
bottleneck engine; PE has slack).
"""

import numpy as np
import ml_dtypes

B, T, C, H = 4, 2048, 1024, 16  # noqa
D = C // H          # 64
HPC = H // 2        # 8 heads per core
CG = C // 2         # 512 channels per head group
P = 128
WS = 32.0           # host-side W_qkv scale for fp8 range
SCALE = float(C) ** -0.5

_compiled = {}


def _build(t=T):
    import concourse.bacc as bacc
    import concourse.tile as tile
    import concourse.mybir as mybir

    f32 = mybir.dt.float32
    bf16 = mybir.dt.bfloat16
    f8 = mybir.dt.float8e4
    DR = mybir.MatmulPerfMode.DoubleRow
    Exp = mybir.ActivationFunctionType.Exp

    KT = C // P            # 8 contraction tiles over C
    KP = KT // 2           # 4 DoubleRow contraction pairs
    TT = t // P            # token tiles of 128
    QC = t // 512          # q chunks of 512
    VB = CG // P           # 4 att channel blocks (= head pairs)
    SCALE2 = SCALE / (WS * WS)

    nc = bacc.Bacc("TRN2", target_bir_lowering=False, debug=False,
                   num_devices=8)

    xf8_d = nc.dram_tensor("xf8", [C, t], f8, kind="ExternalInput")
    xf8l_d = nc.dram_tensor("xf8l", [C, t], f8, kind="ExternalInput")
    wkq_d = nc.dram_tensor("wkq", [C, C], f8, kind="ExternalInput")
    wvh_d = nc.dram_tensor("wvh", [C, CG], f8, kind="ExternalInput")
    wvl_d = nc.dram_tensor("wvl", [C, CG], f8, kind="ExternalInput")
    wp_d = nc.dram_tensor("wp", [CG, C], bf16, kind="ExternalInput")
    y_d = nc.dram_tensor("y", [C, t], f32, kind="ExternalOutput")

    with tile.TileContext(nc) as tc:
        with (
            tc.tile_pool(name="persist", bufs=1) as persist,
            tc.tile_pool(name="psS", bufs=2, space="PSUM") as psS,
            tc.tile_pool(name="avP", bufs=2, space="PSUM") as avP,
            tc.tile_pool(name="qpP", bufs=2, space="PSUM") as qpP,
            tc.tile_pool(name="ptP", bufs=26) as ptP,
            tc.tile_pool(name="rcP", bufs=4) as rcP,
            tc.tile_pool(name="atP", bufs=3) as atP,
            tc.tile_pool(name="yP", bufs=3) as yP,
        ):
            xf8 = persist.tile([P, KP, 2, t], f8)
            xf8l = persist.tile([P, KP, 2, t], f8)
            wkq_sb = persist.tile([P, KP, 2, C], f8)
            wvh_sb = persist.tile([P, KP, 2, CG], f8)
            wvl_sb = persist.tile([P, KP, 2, CG], f8)
            wp_sb = persist.tile([P, VB, C], bf16)
            # kqT[p, side(k/q), quad, s2, tok]
            kqT = persist.tile([P, 2, 2, 2, t], f8)
            v_aug = persist.tile([P, TT, HPC * (D + 1)], bf16)
            att = persist.tile([P, VB, t], bf16)

            # PE warm-up: dependency-free matmuls run during the input-DMA
            # window so the clock ramp is complete when real work starts.
            wu_a = persist.tile([P, P], bf16)
            wu_b = persist.tile([P, 512], bf16)
            nc.vector.memset(wu_a, 0.0)
            nc.vector.memset(wu_b, 0.0)
            for _ in range(14):
                wps = qpP.tile([P, 512], f32, name="wups", tag="qp", bufs=2)
                nc.tensor.matmul(wps, lhsT=wu_a, rhs=wu_b,
                                 start=True, stop=True,
                                 skip_group_check=True)

            # ---- loads, ordered for earliest compute start: wkq + x chunk 0
            # unblock the first kq units / S tiles; wv + xf8l unblock v units
            # (first AV); wp is only needed by the first projection.
            xf8_r = xf8_d.ap().rearrange("(kp s p) t -> p kp s t", p=P, s=2)
            xf8l_r = xf8l_d.ap().rearrange(
                "(kp s p) t -> p kp s t", p=P, s=2)
            wkq_r = wkq_d.ap().rearrange("(kp s p) m -> p kp s m",
                                         p=P, s=2)
            # startup (quad 0, chunk 0) needs wkq blocks 0,1 (cols 0:256)
            # and 4,5 (cols 512:768) plus x chunk 0; v units for chunk c
            # need only the token-chunk-c slices of xf8/xf8l
            nc.sync.dma_start(wkq_sb[:, :, :, 0:512], wkq_r[:, :, :, 0:512])
            nc.sync.dma_start(xf8[:, :, :, 0:512], xf8_r[:, :, :, 0:512])
            nc.sync.dma_start(wkq_sb[:, :, :, 512:1024],
                              wkq_r[:, :, :, 512:1024])
            nc.sync.dma_start(
                wvh_sb, wvh_d.ap().rearrange("(kp s p) m -> p kp s m",
                                             p=P, s=2))
            nc.sync.dma_start(
                wvl_sb, wvl_d.ap().rearrange("(kp s p) m -> p kp s m",
                                             p=P, s=2))
            nc.sync.dma_start(xf8l[:, :, :, 0:512], xf8l_r[:, :, :, 0:512])
            nc.sync.dma_start(xf8[:, :, :, 512:t], xf8_r[:, :, :, 512:t])
            nc.sync.dma_start(xf8l[:, :, :, 512:t], xf8l_r[:, :, :, 512:t])
            nc.sync.dma_start(
                wp_sb, wp_d.ap().rearrange("(kt p) m -> p kt m", p=P))
            # only the ones-columns need initialising; v columns are written
            # by the v units
            nc.vector.memset(
                v_aug.rearrange("p tt (h e) -> p tt h e", e=D + 1)[:, :, :, D],
                1.0)

            # column position of block mb in the host-reordered wkq
            WKQ_POS = {0: 0, 1: 1, 4: 2, 5: 3, 2: 4, 3: 5, 6: 6, 7: 7}

            def emit_kq_unit(mb, c, on_act=False):
                # mb = (side, quad, s2) flat block index 0..7
                side, rem = divmod(mb, 4)
                quad, s2 = divmod(rem, 2)
                pos = WKQ_POS[mb]
                ps = qpP.tile([P, 512], f32, name="pskq", tag="qp", bufs=2)
                for kp in range(KP):
                    nc.tensor.matmul(
                        ps,
                        lhsT=wkq_sb[:, kp, :, pos * P:(pos + 1) * P],
                        rhs=xf8[:, kp, :, c * 512:(c + 1) * 512],
                        start=(kp == 0), stop=(kp == KP - 1),
                        perf_mode=DR, skip_group_check=True)
                out = kqT[:, side, quad, s2, c * 512:(c + 1) * 512]
                if on_act:
                    # startup copies run while ACT is otherwise idle,
                    # keeping the serial DVE copy chain off the first-S path
                    nc.scalar.copy(out, ps)
                else:
                    nc.vector.tensor_copy(out, ps)

            def emit_v_unit(tt):
                ps = qpP.tile([P, CG], f32, name="psv", tag="qp", bufs=2)
                terms = [(xf8, wvh_sb), (xf8l, wvh_sb), (xf8, wvl_sb)]
                for ti, (xs, ws) in enumerate(terms):
                    for kp in range(KP):
                        nc.tensor.matmul(
                            ps,
                            lhsT=xs[:, kp, :, tt * P:(tt + 1) * P],
                            rhs=ws[:, kp, :, :],
                            start=(ti == 0 and kp == 0),
                            stop=(ti == 2 and kp == KP - 1),
                            perf_mode=DR, skip_group_check=True)
                nc.vector.tensor_copy(
                    v_aug[:, tt, :].rearrange(
                        "p (h e) -> p h e", e=D + 1)[:, :, 0:D],
                    ps.rearrange("p (h d) -> p h d", d=D))

            def emit_proj_unit(mb, c):
                ps = qpP.tile([P, 512], f32, name="psp", tag="qp", bufs=2)
                for kt in range(VB):
                    nc.tensor.matmul(
                        ps,
                        lhsT=wp_sb[:, kt, mb * P:(mb + 1) * P],
                        rhs=att[:, kt, c * 512:(c + 1) * 512],
                        start=(kt == 0), stop=(kt == VB - 1),
                        skip_group_check=True)
                yt = yP.tile([P, 512], f32)
                if c == QC - 1 and mb % 2 == 0:
                    # half the last row's copies run on the tail-idle ACT
                    # engine so the two copy lanes pipeline the projection
                    nc.scalar.copy(yt, ps)
                else:
                    nc.vector.tensor_copy(yt, ps)
                nc.sync.dma_start(
                    y_d[mb * P:(mb + 1) * P, c * 512:(c + 1) * 512], yt)

            def emit_s_phase(hp, c, fuse_av=False):
                """S + exp (+ causal mask) for chunk (hp, c).  Returns a
                closure emitting the AV/normalise/transpose phase, so the
                main loop can software-pipeline: S of chunk n+1 is emitted
                before AV of chunk n, keeping the ACT engine fed while the
                PE runs AV and filler units.  With fuse_av (final chunk),
                each AV qs-group is emitted right after its diagonal tile so
                the kernel tail shrinks; returns None."""
                quad = hp // 2
                lanes = (2 * (hp % 2), 2 * (hp % 2) + 1)
                heads = (2 * hp, 2 * hp + 1)

                def s_dr(out_ap, hi, j, w):
                    a = lanes[hi]
                    nc.tensor.matmul(
                        out_ap,
                        lhsT=kqT[32 * a:32 * a + 32, 0, quad, :,
                                 j * P:(j + 1) * P],
                        rhs=kqT[32 * a:32 * a + 32, 1, quad, :,
                                (c + 1) * 512 - w:(c + 1) * 512],
                        start=True, stop=True,
                        perf_mode=DR, skip_group_check=True,
                        tile_position=(32 * a, 0))

                # pt_slices[hi][j] -> AP covering q cols [off_j, 512) of the
                # exp'd S^T tile for (head hi, k-tile j), plus its offset
                pt_slices = [[None] * (4 * c + 4) for _ in range(2)]

                # full j-tiles, processed in (j, j+1) pairs per head
                for jp in range(2 * c):
                    for hi in range(2):
                        st = psS.tile([P, 2, 512], f32, name="st", tag="st")
                        for u in range(2):
                            s_dr(st[:, u, :], hi, 2 * jp + u, 512)
                        pt = ptP.tile([P, 2, 512], bf16, name="pt", tag="pt")
                        nc.scalar.activation(pt, st, Exp, scale=SCALE2)
                        for u in range(2):
                            pt_slices[hi][2 * jp + u] = (pt[:, u, :], 0)
                def emit_av_group(av, hi, qs):
                    h = heads[hi]
                    njs = 4 * c + qs + 1
                    for j in range(njs):
                        pap, off = pt_slices[hi][j]
                        lo = qs * P - off
                        nc.tensor.matmul(
                            av[:, qs, :],
                            lhsT=pap[:, lo:lo + P],
                            rhs=v_aug[:, j,
                                      h * (D + 1):(h + 1) * (D + 1)],
                            start=(j == 0), stop=(j == njs - 1),
                            skip_group_check=True)

                def norm_store(at_q, avs):
                    for hi in range(2):
                        rc = rcP.tile([P, 4], f32)
                        nc.vector.reciprocal(rc, avs[hi][:, :, D])
                        nc.vector.tensor_mul(
                            at_q[:, :, hi, :], avs[hi][:, :, 0:D],
                            rc[:, :, None].broadcast_to([P, 4, D]))
                    # blocked transpose: [128q, (qs hi d)] -> [128, qs, q]
                    nc.sync.dma_start(
                        att[:, hp, c * 512:(c + 1) * 512].rearrange(
                            "p (a b) -> p a b", b=P),
                        at_q, transpose=True)

                if fuse_av:
                    avs = [avP.tile([P, 4, D + 1], f32, name=f"av{hi}",
                                    tag="av") for hi in range(2)]

                # diagonal j-tiles: both heads share a tile per dj;
                # dj2 (w=256) and dj3 (w=128) are packed side by side in one
                # tile (384 f32 < one PSUM bank) sharing a single exp
                for dj, dj2 in ((0, None), (1, None), (2, 3)):
                    j = 4 * c + dj
                    w = 512 - P * dj
                    st = psS.tile([P, 2, 512], f32, name="std", tag="st")
                    for hi in range(2):
                        s_dr(st[:, hi, 0:w], hi, j, w)
                    wtot = w
                    if dj2 is not None:
                        w2 = 512 - P * dj2
                        for hi in range(2):
                            s_dr(st[:, hi, w:w + w2], hi, 4 * c + dj2, w2)
                        wtot = w + w2
                    pt = ptP.tile([P, 2, 512], bf16, name="ptd", tag="pt")
                    nc.scalar.activation(pt[:, :, 0:wtot], st[:, :, 0:wtot],
                                         Exp, scale=SCALE2)
                    nc.gpsimd.affine_select(
                        pt[:, :, 0:P], pt[:, :, 0:P],
                        pattern=[[0, 2], [1, P]],
                        compare_op=mybir.AluOpType.is_ge,
                        fill=0.0, base=0, channel_multiplier=-1)
                    for hi in range(2):
                        pt_slices[hi][j] = (pt[:, hi, 0:w], P * dj)
                    if dj2 is not None:
                        nc.gpsimd.affine_select(
                            pt[:, :, w:w + P], pt[:, :, w:w + P],
                            pattern=[[0, 2], [1, P]],
                            compare_op=mybir.AluOpType.is_ge,
                            fill=0.0, base=0, channel_multiplier=-1)
                        for hi in range(2):
                            pt_slices[hi][4 * c + dj2] = (
                                pt[:, hi, w:w + w2], P * dj2)
                    if fuse_av:
                        for hi in range(2):
                            emit_av_group(avs[hi], hi, dj)

                if fuse_av:
                    at_q = atP.tile([P, 4, 2, D], bf16)
                    norm_store(at_q, avs)
                    return None

                def av_phase():
                    at_q = atP.tile([P, 4, 2, D], bf16)
                    avs = []
                    for hi in range(2):
                        av = avP.tile([P, 4, D + 1], f32, name=f"av{hi}",
                                      tag="av")
                        avs.append(av)
                        for qs in range(4):
                            emit_av_group(av, hi, qs)
                    norm_store(at_q, avs)
                return av_phase

            # ---- startup: just enough for attn(hp=0, c=0); the v
            # units wait on the xf8l load, so they go in the fill queue
            # (needed only by the AVs, which the row-0 lag defers)
            for mb in (0, 1, 4, 5):        # (k, quad0, s2=0/1), (q, quad0, ...)
                emit_kq_unit(mb, 0, on_act=True)

            # Remaining kq/v units, emitted as PE filler between attention
            # chunks.  Tile discovers dependencies from TRACE order, so a
            # producer MUST be emitted before its first consumer chunk; each
            # fill carries the global chunk index (ci = 4*c + hp) it is first
            # needed by.
            def cdiv(a, b):
                return -(-a // b)

            fills = []
            for tt in range(4):
                # spread the first v units (blocked on the xf8l load) across
                # the row-0 chunk boundaries; all are ready before AV(0,0),
                # which the row-0 lag defers past S(3,0)
                fills.append(((min(tt + 1, 3), 1 + (tt == 3)), ("v", tt)))
            for tt in range(4, TT):
                # attn(*, c) AV reads v tiles tt <= 4c+3 (exact need: the
                # early v units would stall the PE on the xf8l/wv loads)
                fills.append(
                    ((4 * max(0, cdiv(tt - 3, 4)), 1), ("v", tt)))
            for side in range(2):
                for quad in range(2):
                    for s2 in range(2):
                        mb = side * 4 + quad * 2 + s2
                        for cc in range(QC):
                            if mb in (0, 1, 4, 5) and cc == 0:
                                continue
                            # k side chunk cc needed by attn(2*quad, c>=cc);
                            # q side chunk cc needed by attn(2*quad, cc)
                            need = 4 * cc + 2 * quad
                            if cc == 0 and quad == 1:
                                need = 1   # wkq half 2 lands early; emit
                                           # before the first v fill
                            fills.append(((need, 0), ("kq", mb, cc)))
            fills.sort(key=lambda f: f[0])

            nchunks = QC * 4
            emitted = 0

            def emit_fills(upto):
                nonlocal emitted
                while emitted < min(upto, len(fills)):
                    _, f = fills[emitted]
                    if f[0] == "kq":
                        emit_kq_unit(f[1], f[2])
                    else:
                        emit_v_unit(f[1])
                    emitted += 1

            # Software-pipelined schedule: S/exp of chunk n+1 is emitted
            # before the AV of chunk n (row 0 keeps all four S-phases ahead
            # so ACT covers the v-unit load window); projection of row c is
            # spread across the AV slots of row c+1.
            pending_av = []      # FIFO of av_phase closures
            pending_proj = []    # FIFO of (mb, c) projection units
            for c in range(QC):
                for hp in range(4):
                    ci = 4 * c + hp
                    # everything this chunk reads must already be emitted
                    while emitted < len(fills) and fills[emitted][0][0] <= ci:
                        emit_fills(emitted + 1)
                    pending_av.append(emit_s_phase(hp, c))
                    # lag taper: row 0 keeps all four S-phases ahead of the
                    # first AV (covers the v-unit load window), row 1 drains
                    # the backlog gradually, steady state keeps one chunk of
                    # S/exp in flight ahead of AV
                    lag = (4 if c == 0 else
                           max(1, 4 - hp) if c == 1 else 1)
                    while len(pending_av) > lag:
                        pending_av.pop(0)()
                        # projection of row cc is paced two rows behind (the
                        # ACT-slack rows); the last row takes double rate
                        nproj = (4 if c == QC - 1 else
                                 3 if c == QC - 2 else 2)
                        for _ in range(nproj):
                            if pending_proj and (
                                    pending_proj[0][1] <= c - 2
                                    or (c == QC - 2
                                        and pending_proj[0][1] <= c - 1)
                                    or c == QC - 1):
                                mb, cc = pending_proj.pop(0)
                                emit_proj_unit(mb, cc)
                    emit_fills(((ci + 3) * len(fills)) // nchunks)
                pending_proj.extend((mb, c) for mb in range(C // P))
            emit_fills(len(fills))
            while pending_av:
                pending_av.pop(0)()
            for mb, cc in pending_proj:
                emit_proj_unit(mb, cc)

    nc.compile()
    return nc


def _get_compiled(t=T):
    if t not in _compiled:
        _compiled[t] = _build(t)
    return _compiled[t]


def make_in_maps(x, W_qkv, W_proj):
    bf = ml_dtypes.bfloat16
    f8 = ml_dtypes.float8_e4m3
    x = np.asarray(x, dtype=np.float32)
    W_qkv = np.asarray(W_qkv, dtype=np.float32)
    W_proj = np.asarray(W_proj, dtype=np.float32)

    in_maps = []
    for core in range(8):
        b, g = core // 2, core % 2
        xT = np.ascontiguousarray(x[b].T)          # [C, T]
        # wkq column order: block (side, quad, s2): lane-major 32-channel
        # slices of heads 8g+4*quad..+3, d-range [32*s2, 32*s2+32)
        cols = []
        for side, quad in ((0, 0), (1, 0), (0, 1), (1, 1)):
            base = side * C
            for s2 in range(2):
                for lq in range(4):
                    h = 8 * g + 4 * quad + lq
                    st = base + h * D + s2 * 32
                    cols.append(np.arange(st, st + 32))
        cols = np.concatenate(cols)
        xf8 = xT.astype(f8)
        wvs = np.ascontiguousarray(
            W_qkv[:, 2 * C + g * CG:2 * C + (g + 1) * CG]) * WS
        wvh = wvs.astype(f8)
        in_maps.append({
            "xf8": xf8,
            "xf8l": (xT - xf8.astype(np.float32)).astype(f8),
            "wkq": (W_qkv[:, cols] * WS).astype(f8),
            "wvh": wvh,
            "wvl": (wvs - wvh.astype(np.float32)).astype(f8),
            # v (and hence att) carries the extra WS factor; fold the
            # inverse into the projection weights
            "wp": np.ascontiguousarray(
                W_proj[g * CG:(g + 1) * CG, :] / WS).astype(bf),
        })
    return in_maps


def _run_axon_nodonate(nc, in_maps, n_cores=8):
    """Execute via PJRT/shard_map WITHOUT output-buffer donation.

    bass2jax.run_bass_via_pjrt donates the zero output operands; under the
    axon transport that donation intermittently corrupts multi-core results.
    This kernel writes every element of its output, so donation is not
    needed for correctness -- pass non-donated zero operands instead.
    """
    import jax
    from jax.sharding import Mesh, PartitionSpec
    from jax.experimental.shard_map import shard_map
    import concourse.mybir as mybir
    from concourse.bass2jax import _bass_exec_p, install_neuronx_cc_hook

    install_neuronx_cc_hook()
    in_names, out_names, out_avals = [], [], []
    for alloc in nc.m.functions[0].allocations:
        if not isinstance(alloc, mybir.MemoryLocationSet):
            continue
        name = alloc.memorylocations[0].name
        if alloc.kind == "ExternalInput":
            in_names.append(name)
        elif alloc.kind == "ExternalOutput":
            out_names.append(name)
            out_avals.append(jax.core.ShapedArray(
                tuple(alloc.tensor_shape), mybir.dt.np(alloc.dtype)))
    n_params = len(in_names)
    all_names = in_names + out_names
    pid_name = nc.partition_id_tensor.name if nc.partition_id_tensor else None

    def _body(*args):
        return tuple(_bass_exec_p.bind(
            *args,
            out_avals=tuple(out_avals),
            in_names=tuple(all_names),
            out_names=tuple(out_names),
            lowering_input_output_aliases=(),
            sim_require_finite=True,
            sim_require_nnan=True,
            nc=nc,
        ))

    devices = jax.devices()[:n_cores]
    mesh = Mesh(np.asarray(devices), ("core",))
    fn = jax.jit(
        shard_map(_body, mesh=mesh,
                  in_specs=(PartitionSpec("core"),) * (n_params + len(out_names)),
                  out_specs=(PartitionSpec("core"),) * len(out_names),
                  check_rep=False),
        keep_unused=True)
    concat_in = [
        np.concatenate([
            np.asarray(in_maps[c].get(
                nm, np.array([[c]], dtype=np.uint32) if nm == pid_name
                else None))
            for c in range(n_cores)], 0)
        for nm in in_names
    ]
    concat_zeros = [
        np.zeros((n_cores * a.shape[0], *a.shape[1:]), a.dtype)
        for a in out_avals
    ]
    out = fn(*concat_in, *concat_zeros)
    return [
        {nm: np.asarray(out[i]).reshape(n_cores, *out_avals[i].shape)[c]
         for i, nm in enumerate(out_names)}
        for c in range(n_cores)
    ]


def kernel(x, W_qkv, W_proj, _trace=False):
    from concourse._compat import axon_active

    nc = _get_compiled()
    in_maps = make_in_maps(x, W_qkv, W_proj)
    if axon_active():
        results = _run_axon_nodonate(nc, in_maps)
    else:
        import concourse.bass_utils as bass_utils
        res = bass_utils.run_bass_kernel_spmd(
            nc, in_maps, core_ids=list(range(8)), trace=_trace)
        if _trace:
            kernel.last_results = res
        results = res.results
    y = np.zeros((B, T, C), np.float32)
    for core in range(8):
        y[core // 2] += results[core]["y"].T
    return y
